# revision 54
# baseline (speedup 1.0000x reference)
"""Trainium2 Bass kernel for nn_CrossLayerAttention_309237645906.

Reference computation (B=2, SQ=SK=2048, H=2048, NH=16, HD=128, fp32):
    q = hidden @ w_q.T + b_q                     -> [B, NH, SQ, HD]
    scores = mask + scale * q @ k                (k given as [B*NH, HD, SK])
    probs = softmax(scores)                      (fp32)
    out = (probs @ v)                            -> [B, SQ, H]
    y = out @ w_proj.T + b_proj

Causal fast path (used when the mask is exactly the standard causal mask):

Sharding: 8 cores = (batch b = c//4) x (strided query set s = c%4: local
query column i <-> global row 4*i + s). The stride-4 mapping makes the
causal work profile IDENTICAL on every core (required: one SPMD program),
and exactly ideal: for key-tile jt only query cols [32*jt, 512) can be
unmasked, so scores/pv matmuls run width W(jt) = 512 - 32*jt
(sum_j W = 4352 = the causal optimum). Only a 32-col "band" at the left
edge of each j-tile straddles the diagonal; a per-core [128, 32]
multiplicative 0/1 mask (same for every jt and head) is applied to the
exp output on the Pool engine (exact: p = exp * {0,1}).

All matmul operands are bf16 (full PE rate, half DMA bytes); accumulation
is f32 in PSUM; softmax runs in f32/bf16 mixed (~3e-3 max rel err, well
under the 2e-2 gate). Softmax denominator: p-tiles are summed on DVE into
S [128, 2, 512] (bf16 2x mode); Pool partition_all_reduce gives Z on all
partitions, DVE takes 1/Z and scales — the PE does no normalization work.
Heads are processed in pairs so exp/recip/band ops cover two heads per
instruction, and the narrow tail j-tiles (8..15) are packed two per PSUM
tile. The output projection is split: pass A (contraction k<8, or less at
earlier pairs) is injected one matmul at a time into phase-2's idle PE
slots and staged in SBUF (yA); phase 3 finishes with pass B + bias (bias
folded in as a rank-1 matmul) and a single DVE add.

Non-causal masks fall back to the general f32r kernel (build_general).
"""

import sys

sys.path.insert(0, "/opt/trn_rl_repo")

import numpy as np

import concourse.bacc as bacc
import concourse.bass as bass
import concourse.bass_isa as bass_isa
import concourse.mybir as mybir
import concourse.tile as tile
from concourse.bass_utils import run_bass_kernel_spmd

F32 = mybir.dt.float32
F32R = mybir.dt.float32r
BF16 = mybir.dt.bfloat16

B, SQ, SK, H, NH = 2, 2048, 2048, 2048, 16
HD = H // NH  # 128
ROWS = 512            # query rows per core
NCORES = 8
KT = H // 128         # 16 contraction tiles for the projections
JT = SK // 128        # 16 key tiles
IT = ROWS // 128      # 4 query 128-tiles per core
SCALE = 1.0 / float(np.sqrt(HD))
NEG = -1e9
MULT = mybir.AluOpType.mult
ADD = mybir.AluOpType.add
EXP = mybir.ActivationFunctionType.Exp
IDENT = mybir.ActivationFunctionType.Identity


def build_causal(cfg=None):
    """Causal-mask kernel, bf16 matmuls, strided query sharding."""
    cfg = {**dict(kvb=4, ppb=10, scb=2, opb=4, rbb=3, yb=3, wqb=2,
                  norm_defer=1, pd=4, pump=0), **(cfg or {})}
    nc = bacc.Bacc()

    xT = nc.dram_tensor("xT", [H, ROWS], BF16, kind="ExternalInput")
    wqT = nc.dram_tensor("wqT", [H, H], BF16, kind="ExternalInput")
    bq = nc.dram_tensor("bq", [H, 1], F32, kind="ExternalInput")
    key = nc.dram_tensor("key", [NH, HD, SK], BF16, kind="ExternalInput")
    vR = nc.dram_tensor("vR", [NH, 128, SK], BF16, kind="ExternalInput")
    band = nc.dram_tensor("band", [128, 64], BF16, kind="ExternalInput")
    bandN = nc.dram_tensor("bandN", [128, 256], BF16, kind="ExternalInput")
    wpT = nc.dram_tensor("wpT", [H, H], BF16, kind="ExternalInput")
    bp1 = nc.dram_tensor("bp1", [1, H], BF16, kind="ExternalInput")
    Y = nc.dram_tensor("Y", [ROWS, H], F32, kind="ExternalOutput")

    with tile.TileContext(nc) as tc:
        with tc.tile_pool(name="res", bufs=1) as res:
            qT_all = res.tile([128, KT, ROWS], BF16)
            attnT_all = res.tile([128, NH, ROWS], BF16)
            # pass-A staging for the 8 output-proj chunks computed during
            # phase 2 (k 0..7 partial sums, flushed from PSUM)
            yA = res.tile([128, 8, 512], F32)
            bq_all = res.tile([128, KT, 1], F32)
            band_sb = res.tile([128, 2, 32], BF16)
            bandN_sb = res.tile([128, 2, 2, 64], BF16)
            bp1_sb = res.tile([1, H], BF16)
            ones_sb = res.tile([128, 1], BF16)
            nc.vector.memset(ones_sb, 1.0)
            ones1_sb = res.tile([1, 128], BF16)
            nc.vector.memset(ones1_sb, 1.0)

            def load_consts():
                nc.sync.dma_start(bq_all,
                                  bq[:, :].rearrange("(t p) x -> p t x", p=128))
                nc.sync.dma_start(band_sb,
                                  band[:, :].rearrange("p (u c) -> p u c", c=32))
                nc.sync.dma_start(bandN_sb, bandN[:, :].rearrange(
                    "p (u j c) -> p u j c", j=2, c=64))
                nc.sync.dma_start(bp1_sb, bp1[:, :])

            # PSUM: sc 2bufs x 2banks + op 3 + zp 1 = 8 banks
            ps = tc.alloc_tile_pool(name="ps", bufs=1, space="PSUM")
            # long-lived SBUF pools
            kv = tc.alloc_tile_pool(name="kv", bufs=cfg["kvb"])
            pp = tc.alloc_tile_pool(name="pp", bufs=cfg["ppb"])
            sS = tc.alloc_tile_pool(name="sS", bufs=2)
            rr = tc.alloc_tile_pool(name="rr", bufs=2)
            ypo = tc.alloc_tile_pool(name="ypo", bufs=cfg["yb"])

            def kv_load(h):
                k_sb = kv.tile([128, SK], BF16, tag="k", name=f"k{h}")
                nc.sync.dma_start(k_sb, key[h, :, :])
                v_sb = kv.tile([128, SK], BF16, tag="v", name=f"v{h}")
                nc.sync.dma_start(v_sb, vR[h, :, :])
                return k_sb, v_sb

            # output-projection weights, split by contraction half:
            # wpA = k 0..7 (used by pass-A chunks injected into phase 2),
            # wpB = k 8..15 (pass B / full chunks in phase 3)
            wpAs, wpBs = {}, {}

            def wp_load(oc, half):
                tag = "wpA" if half == 0 else "wpB"
                wp_sb = wpp.tile([128, 8, 512], BF16, tag=tag,
                                 name=f"{tag}{oc}")
                wp_ap = wpT[1024 * half:1024 * (half + 1),
                            512 * oc:512 * (oc + 1)].rearrange(
                    "(k p) o -> p k o", p=128)
                nc.sync.dma_start(wp_sb, wp_ap)
                return wp_sb

            # ---- phase 2: attention, head pairs g -> heads (2g, 2g+1) ----
            # Z = colsum(p) via Pool partition_all_reduce on S, then
            # rb = 1/Z (DVE) and attnT = op * rb (DVE). No PE involvement.
            norm_pend = []

            def do_norm(g, S, ops):
                Zb = rr.tile([128, 2, 512], F32, tag="Zb", bufs=cfg["rbb"],
                             name=f"Zb{g}")
                nc.gpsimd.partition_all_reduce(Zb, S, 128, bass_isa.ReduceOp.add)
                rb = rr.tile([128, 2, 512], F32, tag="rb", bufs=cfg["rbb"],
                             name=f"rb{g}")
                nc.vector.reciprocal(rb, Zb)
                for u in range(2):
                    h = 2 * g + u
                    nc.vector.tensor_tensor(attnT_all[:, h, :], ops[u],
                                            rb[:, u, :], op=MULT)

            chunk_state = {}
            chunk_k0 = {}  # c -> first k left for pass B

            def inject_mm(c, k, klast):
                # one matmul of pass-A chunk c (psy = sum_{k<=klast} ...),
                # spread across phase-2 steps to fit the fragmented PE idle
                oc, it = c // 4, c % 4
                if c not in chunk_state:
                    chunk_state[c] = ps.tile([128, 512], F32, tag="op",
                                             bufs=cfg["opb"], name=f"pyA{c}")
                psy = chunk_state[c]
                nc.tensor.matmul(psy, attnT_all[:, k, 128 * it:128 * (it + 1)],
                                 wpAs[oc][:, k, :], start=(k == 0),
                                 stop=(k == klast), skip_group_check=True)
                if k == klast:
                    nc.vector.tensor_scalar(yA[:, c, :], psy, 1.0, None,
                                            op0=MULT)
                    del chunk_state[c]
                    chunk_k0[c] = klast + 1

            def pair_gen(g):
                h0, h1 = 2 * g, 2 * g + 1
                if h0 not in kvs:
                    kvs[h0] = kv_load(h0)
                if h1 not in kvs:
                    kvs[h1] = kv_load(h1)
                if 2 * (g + 1) not in kvs and g + 1 < 8:  # prefetch next pair
                    kvs[2 * (g + 1)] = kv_load(2 * (g + 1))
                    kvs[2 * (g + 1) + 1] = kv_load(2 * (g + 1) + 1)
                if g == 1:
                    wpAs[0] = wp_load(0, 0)
                elif g == 3:
                    wpAs[1] = wp_load(1, 0)
                elif g == 6:
                    wpBs[0] = wp_load(0, 1)
                elif g == 7:
                    wpBs[1] = wp_load(1, 1)
                k0, v0 = kvs.pop(h0)
                k1, v1 = kvs.pop(h1)
                ks, vs = (k0, k1), (v0, v1)

                S = sS.tile([128, 2, 512], BF16, tag="S", name=f"S{g}")
                op0 = ps.tile([128, 512], F32, tag="op", bufs=cfg["opb"],
                              name=f"op{h0}")
                op1 = ps.tile([128, 512], F32, tag="op", bufs=cfg["opb"],
                              name=f"op{h1}")
                ops = (op0, op1)
                pend = []  # consume-callbacks, one step late

                def consume_wide(jt, p_sb):
                    b0 = 32 * jt
                    W = 512 - b0
                    for u in range(2):
                        nc.tensor.matmul(ops[u][:, b0:], vs[u][:, 128 * jt:128 * (jt + 1)],
                                         p_sb[:, u, :W],
                                         start=(jt == 0), stop=False,
                                         skip_group_check=True)
                    if jt == 0:
                        nc.vector.tensor_scalar(S, p_sb, 1.0, None, op0=MULT)
                    else:
                        nc.vector.tensor_tensor(S[:, :, b0:], S[:, :, b0:],
                                                p_sb[:, :, :W], op=ADD)

                def consume_narrow(jt0, W0, p_sb):
                    b0 = 32 * jt0
                    for u in range(2):
                        for jj in range(2):
                            nc.tensor.matmul(
                                ops[u][:, b0:b0 + W0],
                                vs[u][:, 128 * (jt0 + jj):128 * (jt0 + jj + 1)],
                                p_sb[:, u, jj, :W0],
                                start=False, stop=(jt0 + jj == JT - 1),
                                skip_group_check=True)
                    for jj in range(2):
                        nc.vector.tensor_tensor(S[:, :, b0:b0 + W0],
                                                S[:, :, b0:b0 + W0],
                                                p_sb[:, :, jj, :W0], op=ADD)

                # wide steps: one j-tile each (jt 0..7); narrow steps: two
                # j-tiles share one 2-bank PSUM tile (jt 8..15)
                for st in range(12):
                    if st < 8:
                        jt = st
                        b0 = 32 * jt
                        W = 512 - b0
                        sc = ps.tile([128, 2, 512], F32, tag="sc", bufs=cfg["scb"],
                                     name=f"sc{g}_{st}")
                        for u in range(2):
                            nc.tensor.matmul(sc[:, u, b0:],
                                             ks[u][:, 128 * jt:128 * (jt + 1)],
                                             qT_all[:, 2 * g + u, b0:],
                                             start=True, stop=True)
                        p_sb = pp.tile([128, 2, 512], BF16, tag="p",
                                       bufs=cfg["ppb"], name=f"p{g}_{st}")
                        nc.scalar.activation(p_sb[:, :, :W], sc[:, :, b0:], EXP,
                                             scale=SCALE)
                        nc.gpsimd.tensor_tensor(p_sb[:, :, 0:32], p_sb[:, :, 0:32],
                                                band_sb, op=MULT)
                        pend.append((consume_wide, (jt, p_sb)))
                    else:
                        jt0 = 8 + 2 * (st - 8)
                        b0 = 32 * jt0
                        W0 = 512 - b0  # both sub-tiles computed at width W0
                        sc = ps.tile([128, 2, 2, 256], F32, tag="sc",
                                     bufs=cfg["scb"], name=f"sc{g}_{st}")
                        for u in range(2):
                            for jj in range(2):
                                nc.tensor.matmul(
                                    sc[:, u, jj, :W0],
                                    ks[u][:, 128 * (jt0 + jj):128 * (jt0 + jj + 1)],
                                    qT_all[:, 2 * g + u, b0:],
                                    start=True, stop=True)
                        p_sb = pp.tile([128, 2, 2, 256], BF16, tag="p",
                                       bufs=cfg["ppb"], name=f"p{g}_{st}")
                        nc.scalar.activation(p_sb[:, :, :, :W0], sc[:, :, :, :W0],
                                             EXP, scale=SCALE)
                        nc.vector.tensor_tensor(p_sb[:, :, :, 0:64],
                                                p_sb[:, :, :, 0:64],
                                                bandN_sb, op=MULT)
                        pend.append((consume_narrow, (jt0, W0, p_sb)))
                    if len(pend) > cfg["pd"]:
                        fn, args = pend.pop(0)
                        fn(*args)
                    if st == cfg["norm_defer"] and norm_pend:
                        do_norm(*norm_pend.pop(0))
                    if g in (2, 3):
                        # two chunks, k limited by normalized heads
                        klast = 3 if g == 2 else 5
                        if 2 <= st <= 2 + klast:
                            inject_mm((g - 2) * 2, st - 2, klast)
                        if 4 <= st <= 4 + klast:
                            inject_mm((g - 2) * 2 + 1, st - 4, klast)
                    elif g >= 4:
                        if 1 <= st <= 8:
                            inject_mm(g, st - 1, 7)
                    yield
                while pend:
                    fn, args = pend.pop(0)
                    fn(*args)
                norm_pend.append((g, S, ops))

            kvs = {}
            gen0 = pair_gen(0)

            # ---- phase 1: q projection (o-chunks of 512, t = o-tile) ----
            with tc.tile_pool(name="p1", bufs=1) as p1, \
                 tc.tile_pool(name="p1w", bufs=cfg["wqb"]) as p1w:
                xT_all = p1.tile([128, KT, ROWS], BF16)
                xT_ap = xT[:, :].rearrange("(t p) i -> p t i", p=128)
                for c in range(4):
                    wq_sb = p1w.tile([128, KT, 512], BF16, tag="wq")
                    wq_ap = wqT[:, 512 * c:512 * (c + 1)].rearrange(
                        "(k p) o -> p k o", p=128)
                    if c == 0:
                        # interleave xT / wq-c0 pieces: first matmuls unblock
                        # after ~0.5MB instead of ~4MB
                        for q0, q1 in [(0, 1), (1, 2), (2, 4), (4, 8),
                                       (8, 12), (12, 16)]:
                            nc.sync.dma_start(xT_all[:, q0:q1, :],
                                              xT_ap[:, q0:q1, :])
                            nc.sync.dma_start(wq_sb[:, q0:q1, :],
                                              wq_ap[:, q0:q1, :])
                            if q0 == 2:
                                load_consts()
                    else:
                        nc.sync.dma_start(wq_sb, wq_ap)
                    if c == 2:
                        for h in range(4):  # prefetch k/v for pairs 0-1
                            kvs[h] = kv_load(h)
                    for u in range(4):
                        t = 4 * c + u
                        psq = ps.tile([128, 512], F32, tag="op", bufs=cfg["opb"],
                                      name=f"psq{t}")
                        for k in range(KT):
                            nc.tensor.matmul(psq, wq_sb[:, k, 128 * u:128 * (u + 1)],
                                             xT_all[:, k, :],
                                             start=(k == 0), stop=(k == KT - 1))
                        if c == 3:
                            # last chunk's bias on DVE: frees ACT so pair-0
                            # exps start sooner at the phase transition
                            nc.vector.tensor_scalar(qT_all[:, t, :], psq,
                                                    bq_all[:, t, :], None,
                                                    op0=ADD)
                        else:
                            nc.scalar.activation(qT_all[:, t, :], psq, IDENT,
                                                 bias=bq_all[:, t, :])
                        if c >= 1 and cfg["pump"]:
                            # pump one attention step of pair 0 between
                            # q-projection tiles: its exp/Z work hides under
                            # phase-1 PE time
                            next(gen0, None)

            wpp = tc.alloc_tile_pool(name="wpp", bufs=2)
            for _ in gen0:
                pass
            for g in range(1, 8):
                for _ in pair_gen(g):
                    pass
            while norm_pend:
                do_norm(*norm_pend.pop(0))

            # ---- phase 3 ----
            # pass B for oc 0,1 (k 8..15 + bias, added to the staged yA),
            # then full chunks for oc 2,3
            wpAs[2] = wp_load(2, 0)
            wpBs[2] = wp_load(2, 1)
            # oc 0: run all four chunks' k<14 first (independent of the
            # last pair's norm chain), then finish k14/15 + bias per chunk.
            # Two chunks borrow idle sc-tag PSUM slots to avoid aliasing
            # pair-7's still-live op accumulators.
            psysB = []
            for it in range(IT):
                c = it
                k0c = chunk_k0[c]
                psy = ps.tile([128, 512], F32,
                              tag=("sc" if it < 2 else "op"),
                              bufs=(cfg["scb"] if it < 2 else cfg["opb"]),
                              name=f"psyB0_{it}")
                for k in range(k0c, 14):
                    wsrc = (wpAs[0][:, k, :] if k < 8
                            else wpBs[0][:, k - 8, :])
                    nc.tensor.matmul(psy, attnT_all[:, k, 128 * it:128 * (it + 1)],
                                     wsrc, start=(k == k0c), stop=False,
                                     skip_group_check=True)
                psysB.append((it, c, psy))
            for it, c, psy in psysB:
                for k in (14, 15):
                    nc.tensor.matmul(psy, attnT_all[:, k, 128 * it:128 * (it + 1)],
                                     wpBs[0][:, k - 8, :], start=False,
                                     stop=False, skip_group_check=True)
                nc.tensor.matmul(psy, ones1_sb, bp1_sb[:, 0:512],
                                 start=False, stop=True, skip_group_check=True)
                y_sb = ypo.tile([128, 512], F32, tag="y", name=f"yB0_{it}")
                nc.vector.tensor_tensor(y_sb, psy, yA[:, c, :], op=ADD)
                nc.sync.dma_start(Y[128 * it:128 * (it + 1), 0:512], y_sb)
            wpAs[3] = wp_load(3, 0)
            wpBs[3] = wp_load(3, 1)
            for oc in range(1, 2):
                for it in range(IT):
                    c = 4 * oc + it
                    k0c = chunk_k0[c]
                    psy = ps.tile([128, 512], F32, tag="op", bufs=cfg["opb"],
                                  name=f"psyB{oc}_{it}")
                    for k in range(k0c, KT):
                        wsrc = (wpAs[oc][:, k, :] if k < 8
                                else wpBs[oc][:, k - 8, :])
                        nc.tensor.matmul(psy, attnT_all[:, k, 128 * it:128 * (it + 1)],
                                         wsrc,
                                         start=(k == k0c), stop=False,
                                         skip_group_check=True)
                    nc.tensor.matmul(psy, ones1_sb,
                                     bp1_sb[:, 512 * oc:512 * (oc + 1)],
                                     start=False, stop=True, skip_group_check=True)
                    y_sb = ypo.tile([128, 512], F32, tag="y", name=f"y{oc}_{it}")
                    nc.vector.tensor_tensor(y_sb, psy, yA[:, c, :], op=ADD)
                    nc.sync.dma_start(
                        Y[128 * it:128 * (it + 1), 512 * oc:512 * (oc + 1)], y_sb)
            for oc in range(2, 4):
                for it in range(IT):
                    psy = ps.tile([128, 512], F32, tag="op", bufs=cfg["opb"],
                                  name=f"psy{oc}_{it}")
                    for k in range(KT):
                        wp_sb = wpAs[oc] if k < 8 else wpBs[oc]
                        nc.tensor.matmul(psy, attnT_all[:, k, 128 * it:128 * (it + 1)],
                                         wp_sb[:, k % 8, :],
                                         start=(k == 0), stop=False,
                                         skip_group_check=True)
                    nc.tensor.matmul(psy, ones1_sb,
                                     bp1_sb[:, 512 * oc:512 * (oc + 1)],
                                     start=False, stop=True, skip_group_check=True)
                    y_sb = ypo.tile([128, 512], F32, tag="y", name=f"y{oc}_{it}")
                    nc.scalar.copy(y_sb, psy)
                    nc.sync.dma_start(
                        Y[128 * it:128 * (it + 1), 512 * oc:512 * (oc + 1)], y_sb)
            wpp.release()
            ypo.release()
            rr.release()
            sS.release()
            pp.release()
            kv.release()
            ps.release()

    nc.compile()
    return nc


def build_general(mm_dt=F32R, mask_dt=F32):
    """General-mask fallback (row-block sharding, f32r matmuls). This is the
    previous-generation kernel, kept for non-causal attention_mask inputs."""
    cfg = dict(kv=2, tp=4, pp=4, p1w=2, scb=4, zpb=1, opb=1, GS=1, wpp=4)
    GS = cfg["GS"]
    nc = bacc.Bacc()

    xT = nc.dram_tensor("xT", [H, ROWS], mm_dt, kind="ExternalInput")
    wqT = nc.dram_tensor("wqT", [H, H], mm_dt, kind="ExternalInput")
    bq = nc.dram_tensor("bq", [H, 1], F32, kind="ExternalInput")
    key = nc.dram_tensor("key", [NH, HD, SK], mm_dt, kind="ExternalInput")
    value = nc.dram_tensor("value", [NH, SK, HD], mm_dt, kind="ExternalInput")
    maskT = nc.dram_tensor("maskT", [SK, ROWS], mask_dt, kind="ExternalInput")
    wpT = nc.dram_tensor("wpT", [H, H], mm_dt, kind="ExternalInput")
    bpB = nc.dram_tensor("bpB", [128, H], F32, kind="ExternalInput")
    onesd = nc.dram_tensor("onesd", [128, 1], mm_dt, kind="ExternalInput")
    ones1d = nc.dram_tensor("ones1d", [1, 128], mm_dt, kind="ExternalInput")
    Y = nc.dram_tensor("Y", [ROWS, H], F32, kind="ExternalOutput")

    with tile.TileContext(nc) as tc:
        with tc.tile_pool(name="res", bufs=1) as res:
            qT_all = res.tile([128, KT, ROWS], mm_dt)
            attnT_all = res.tile([128, NH, ROWS], mm_dt)
            maskT_all = res.tile([128, JT, ROWS], mask_dt)
            bq_all = res.tile([128, KT, 1], F32)
            nc.sync.dma_start(bq_all, bq[:, :].rearrange("(t p) x -> p t x", p=128))
            bpB_all = res.tile([128, H], F32)
            nc.sync.dma_start(bpB_all, bpB[:, :])
            ones_sb = res.tile([128, 1], mm_dt)
            nc.sync.dma_start(ones_sb, onesd[:, :])
            ones1_sb = res.tile([1, 128], mm_dt)
            nc.sync.dma_start(ones1_sb, ones1d[:, :])

            wpp = tc.alloc_tile_pool(name="wpp", bufs=cfg["wpp"])
            kv = tc.alloc_tile_pool(name="kv", bufs=cfg["kv"])
            tp = tc.alloc_tile_pool(name="tp", bufs=cfg["tp"])
            pp = tc.alloc_tile_pool(name="pp", bufs=cfg["pp"])
            ps_s = tc.alloc_tile_pool(name="ps_s", bufs=cfg["scb"], space="PSUM")
            ps_z = tc.alloc_tile_pool(name="ps_z", bufs=cfg["zpb"], space="PSUM")
            ps_o = tc.alloc_tile_pool(name="ps_o", bufs=cfg["opb"], space="PSUM")

            with tc.tile_pool(name="p1", bufs=1) as p1, \
                 tc.tile_pool(name="p1w", bufs=cfg["p1w"]) as p1w, \
                 tc.tile_pool(name="ps_q", bufs=2, space="PSUM") as ps_q:
                xT_all = p1.tile([128, KT, ROWS], mm_dt)
                xT_ap = xT[:, :].rearrange("(t p) i -> p t i", p=128)
                for k in range(KT):
                    nc.sync.dma_start(xT_all[:, k, :], xT_ap[:, k, :])
                wqT_ap = wqT[:, :].rearrange("(a p) o -> p a o", p=128)
                for t in range(KT):
                    w_sb = p1w.tile([128, KT, 128], mm_dt, tag="wq")
                    nc.sync.dma_start(w_sb[:, :KT // 2, :],
                                      wqT_ap[:, :KT // 2, 128 * t:128 * (t + 1)])
                    nc.sync.dma_start(w_sb[:, KT // 2:, :],
                                      wqT_ap[:, KT // 2:, 128 * t:128 * (t + 1)])
                    psq = ps_q.tile([128, ROWS], F32, tag="psq")
                    for k in range(KT):
                        nc.tensor.matmul(psq, w_sb[:, k, :], xT_all[:, k, :],
                                         start=(k == 0), stop=(k == KT - 1))
                    nc.scalar.activation(qT_all[:, t, :], psq, IDENT,
                                         bias=bq_all[:, t, :])

            sm = tc.alloc_tile_pool(name="sm", bufs=2)
            maskT_ap = maskT[:, :].rearrange("(t p) i -> p t i", p=128)
            for j in range(JT):
                nc.sync.dma_start(maskT_all[:, j, :], maskT_ap[:, j, :])
            for h in range(NH):
                k_sbs, v_sbs = [], []
                for hf in range(2):
                    k_sb = kv.tile([128, JT // 2, 128], mm_dt, tag="k",
                                   name=f"k{h}_{hf}")
                    nc.sync.dma_start(
                        k_sb, key[h, :, 1024 * hf:1024 * (hf + 1)]
                        .rearrange("d (a j) -> d a j", j=128))
                    v_sb = kv.tile([128, JT // 2, 128], mm_dt, tag="v",
                                   name=f"v{h}_{hf}")
                    nc.sync.dma_start(
                        v_sb, value[h, 1024 * hf:1024 * (hf + 1), :]
                        .rearrange("(a p) d -> p a d", p=128))
                    k_sbs.append(k_sb)
                    v_sbs.append(v_sb)

                zp = ps_z.tile([1, ROWS], F32, tag="z")
                op = ps_o.tile([128, ROWS], F32, tag="o")
                pend = []

                def consume(gp, p_tile):
                    for uu in range(p_tile.shape[1]):
                        jtc = GS * gp + uu
                        nc.tensor.matmul(op, v_sbs[jtc // 8][:, jtc % 8, :],
                                         p_tile[:, uu, :],
                                         start=(jtc == 0), stop=(jtc == JT - 1))
                        nc.tensor.matmul(zp, ones_sb, p_tile[:, uu, :],
                                         start=(jtc == 0), stop=(jtc == JT - 1))

                for gidx in range(JT // GS):
                    W = ROWS
                    sc = ps_s.tile([128, GS * W], F32, tag="s", name=f"sc{h}_{gidx}")
                    t_sb = tp.tile([128, GS, W], F32, tag="t", name=f"t{h}_{gidx}")
                    for u in range(GS):
                        jt = GS * gidx + u
                        nc.tensor.matmul(sc[:, W * u:W * (u + 1)],
                                         k_sbs[jt // 8][:, jt % 8, :],
                                         qT_all[:, h, :], start=True, stop=True)
                        nc.vector.scalar_tensor_tensor(
                            t_sb[:, u, :], sc[:, W * u:W * (u + 1)],
                            1.0, maskT_all[:, jt, :], MULT, ADD)
                    p_sb = pp.tile([128, GS, W], mm_dt, tag="p", name=f"p{h}_{gidx}")
                    nc.scalar.activation(p_sb, t_sb, EXP, scale=SCALE)
                    pend.append((gidx, p_sb))
                    if len(pend) > 1:
                        consume(*pend.pop(0))
                while pend:
                    consume(*pend.pop(0))

                rc = sm.tile([1, ROWS], mm_dt, tag="rc")
                with nc.allow_low_precision(reason="f32r reciprocal storage"):
                    nc.vector.reciprocal(rc, zp)
                bc = ps_s.tile([128, ROWS], F32, tag="s")
                nc.tensor.matmul(bc, ones1_sb, rc, start=True, stop=True)
                rb = sm.tile([128, ROWS], F32, tag="rb")
                nc.scalar.copy(rb, bc)
                nc.vector.tensor_tensor(attnT_all[:, h, :], op, rb, op=MULT)

            sm.release()
            ps_o.release()
            ps_z.release()
            ps_s.release()
            pp.release()
            tp.release()
            kv.release()

            with tc.tile_pool(name="ypo", bufs=2) as ypo, \
                 tc.tile_pool(name="ps_y", bufs=4, space="PSUM") as ps_y:
                wpT_ap = wpT[:, :].rearrange("(a p) o -> p a o", p=128)
                for half in range(2):
                    o0 = 1024 * half
                    psys = []
                    for it in range(IT):
                        psy = ps_y.tile([128, 1024], F32, tag="y",
                                        name=f"psy{half}_{it}")
                        psys.append(psy)
                    for k in range(KT):
                        wp_sb = wpp.tile([128, 1024], mm_dt, tag="wp")
                        nc.sync.dma_start(wp_sb, wpT_ap[:, k, o0:o0 + 1024])
                        for it in range(IT):
                            att = attnT_all[:, k, 128 * it:128 * (it + 1)]
                            for nb in range(2):
                                nc.tensor.matmul(
                                    psys[it][:, 512 * nb:512 * (nb + 1)],
                                    att, wp_sb[:, 512 * nb:512 * (nb + 1)],
                                    start=(k == 0), stop=(k == KT - 1))
                    for it in range(IT):
                        y_sb = ypo.tile([128, 1024], F32, tag="ysb")
                        nc.vector.tensor_tensor(y_sb, psys[it],
                                                bpB_all[:, o0:o0 + 1024], op=ADD)
                        nc.sync.dma_start(
                            Y[128 * it:128 * (it + 1), o0:o0 + 1024], y_sb)
            wpp.release()

    nc.compile()
    return nc


_CACHE = {}


def _get_nc(mm_dt=None, mask_dt=None, causal=True, cfg=None):
    """test.py compatibility: causal=True returns the bf16 causal kernel
    (mm_dt/mask_dt ignored); causal=False returns the general kernel."""
    if causal:
        ck = ("causal", tuple(sorted((cfg or {}).items())))
        if ck not in _CACHE:
            _CACHE[ck] = build_causal(cfg)
        return _CACHE[ck]
    ck = ("gen", str(mm_dt), str(mask_dt))
    if ck not in _CACHE:
        _CACHE[ck] = build_general(mm_dt if mm_dt is not None else F32R,
                                   mask_dt if mask_dt is not None else F32)
    return _CACHE[ck]


def _is_causal(attention_mask):
    """True if the mask is exactly the standard causal additive mask."""
    m = attention_mask
    if m.shape != (B, 1, SQ, SK):
        return False
    m0 = np.asarray(m[0, 0])
    tri = np.tril(np.ones((SQ, SK), dtype=bool))
    ref = np.where(tri, np.float32(0.0), np.float32(NEG))
    if not np.array_equal(m0, ref):
        return False
    for b in range(1, B):
        if not np.array_equal(np.asarray(m[b, 0]), m0):
            return False
    return True


def _kernel_causal(hidden_states, key, value, w_q, b_q, w_proj, b_proj,
                   _cfg=None, _trace=False):
    import ml_dtypes
    bf16 = ml_dtypes.bfloat16
    nc = _get_nc(causal=True, cfg=_cfg)

    wqT = np.ascontiguousarray(w_q.T).astype(bf16)
    wpT = np.ascontiguousarray(w_proj.T).astype(bf16)
    bq2 = np.ascontiguousarray(b_q[:, None]).astype(np.float32)
    bp1 = np.ascontiguousarray(b_proj[None, :]).astype(bf16)
    key_b = [np.ascontiguousarray(key[b * NH:(b + 1) * NH]).astype(bf16)
             for b in range(B)]
    # vR[h, p, a*HD + d] = v[h, a*128 + p, d]  (4KB contiguous runs per line)
    vR_b = [np.ascontiguousarray(
        value[b].reshape(NH, SK // 128, 128, HD).transpose(0, 2, 1, 3)
        .reshape(NH, 128, SK)).astype(bf16) for b in range(B)]
    inv_scale = np.float32(1.0 / SCALE)

    # band mask: for key-tile jt, col u of the 32-col band (query row
    # 128*jt + 4*u + s, key 128*jt + p): unmasked iff p <= 4*u + s
    p_idx = np.arange(128)[:, None]
    u_idx = np.arange(32)[None, :]

    in_maps = []
    for c in range(NCORES):
        b, s = c // 4, c % 4
        rows = s + 4 * np.arange(ROWS)
        xT_c = np.ascontiguousarray(hidden_states[b, rows, :].T).astype(bf16)
        # multiplicative post-exp masks: 1 keep, 0 drop
        band_c = (p_idx <= 4 * u_idx + s).astype(np.float32)
        band2 = np.ascontiguousarray(
            np.concatenate([band_c, band_c], axis=1)).astype(bf16)
        # bandN [128, (u, jj, 64)]: jj=0 -> [band | 1]; jj=1 -> [0 | band]
        one = np.ones((128, 32), np.float32)
        zer = np.zeros((128, 32), np.float32)
        row = np.concatenate([band_c, one, zer, band_c], axis=1)  # [128, 128]
        bandN_c = np.ascontiguousarray(
            np.concatenate([row, row], axis=1)).astype(bf16)  # [128, 256]
        in_maps.append(dict(
            xT=xT_c, wqT=wqT, bq=bq2, key=key_b[b], vR=vR_b[b],
            band=band2, bandN=bandN_c, wpT=wpT, bp1=bp1,
        ))

    kw = {}
    if _trace:
        kw = dict(trace=True, trace_cores=list(range(NCORES)), stitch_traces=False)
    res = run_bass_kernel_spmd(nc, in_maps, core_ids=list(range(NCORES)), **kw)
    if _trace:
        kernel._last_result = res

    out = np.empty((B, SQ, H), dtype=np.float32)
    for c in range(NCORES):
        b, s = c // 4, c % 4
        rows = s + 4 * np.arange(ROWS)
        out[b, rows, :] = res.results[c]["Y"]
    return out


def _kernel_general(hidden_states, key, value, attention_mask,
                    w_q, b_q, w_proj, b_proj, _mm_dt=F32R):
    nc = _get_nc(_mm_dt, F32, causal=False)
    wqT = np.ascontiguousarray(w_q.T)
    wpT = np.ascontiguousarray(w_proj.T)
    bq2 = np.ascontiguousarray(b_q[:, None]).astype(np.float32)
    bpB = np.ascontiguousarray(
        np.broadcast_to(b_proj[None, :], (128, H))).astype(np.float32)
    key_b = [np.ascontiguousarray(key[b * NH:(b + 1) * NH]) for b in range(B)]
    val_b = [np.ascontiguousarray(value[b]) for b in range(B)]
    inv_scale = np.float32(1.0 / SCALE)

    in_maps = []
    for c in range(NCORES):
        b, sидx = c // 4, c % 4
        rows = np.arange(ROWS * sидx, ROWS * sидx + ROWS)
        xT_c = np.ascontiguousarray(hidden_states[b, rows, :].T)
        maskT_c = np.ascontiguousarray(
            (attention_mask[b, 0, rows, :].T * inv_scale).astype(np.float32))
        in_maps.append(dict(
            xT=xT_c, wqT=wqT, bq=bq2, key=key_b[b], value=val_b[b],
            maskT=maskT_c, wpT=wpT, bpB=bpB,
            onesd=np.ones((128, 1), dtype=np.float32),
            ones1d=np.ones((1, 128), dtype=np.float32),
        ))
    res = run_bass_kernel_spmd(nc, in_maps, core_ids=list(range(NCORES)))
    out = np.empty((B, SQ, H), dtype=np.float32)
    for c in range(NCORES):
        b, sидx = c // 4, c % 4
        rows = np.arange(ROWS * sидx, ROWS * sидx + ROWS)
        out[b, rows, :] = res.results[c]["Y"]
    return out


def kernel(hidden_states, key, value, attention_mask, w_q, b_q, w_proj, b_proj,
           _mm_dt=F32R, _trace=False, _cfg=None):
    hidden_states = np.asarray(hidden_states)
    key = np.asarray(key)
    value = np.asarray(value)
    attention_mask = np.asarray(attention_mask)
    w_q = np.asarray(w_q)
    b_q = np.asarray(b_q)
    w_proj = np.asarray(w_proj)
    b_proj = np.asarray(b_proj)

    if _is_causal(attention_mask):
        return _kernel_causal(hidden_states, key, value, w_q, b_q,
                              w_proj, b_proj, _cfg=_cfg, _trace=_trace)
    return _kernel_general(hidden_states, key, value, attention_mask,
                           w_q, b_q, w_proj, b_proj, _mm_dt=_mm_dt)


BF16_ = BF16  # back-compat alias


if __name__ == "__main__":
    pass


# revision 56
# speedup vs baseline: 1.0067x; 1.0067x over previous
"""Trainium2 Bass kernel for nn_CrossLayerAttention_309237645906.

Reference computation (B=2, SQ=SK=2048, H=2048, NH=16, HD=128, fp32):
    q = hidden @ w_q.T + b_q                     -> [B, NH, SQ, HD]
    scores = mask + scale * q @ k                (k given as [B*NH, HD, SK])
    probs = softmax(scores)                      (fp32)
    out = (probs @ v)                            -> [B, SQ, H]
    y = out @ w_proj.T + b_proj

Causal fast path (used when the mask is exactly the standard causal mask):

Sharding: 8 cores = (batch b = c//4) x (strided query set s = c%4: local
query column i <-> global row 4*i + s). The stride-4 mapping makes the
causal work profile IDENTICAL on every core (required: one SPMD program),
and exactly ideal: for key-tile jt only query cols [32*jt, 512) can be
unmasked, so scores/pv matmuls run width W(jt) = 512 - 32*jt
(sum_j W = 4352 = the causal optimum). Only a 32-col "band" at the left
edge of each j-tile straddles the diagonal; a per-core [128, 32]
multiplicative 0/1 mask (same for every jt and head) is applied to the
exp output (exact: p = exp * {0,1}) — on Pool for the wide steps, on DVE
for the narrow tail steps so the pair-end drain never queues behind the
Pool partition_all_reduce.

All matmul operands are bf16 (full PE rate, half DMA bytes); accumulation
is f32 in PSUM; softmax runs in f32/bf16 mixed (~3e-3 max rel err, well
under the 2e-2 gate). Softmax denominator: p-tiles are summed on DVE into
S [128, 2, 512] (bf16 2x mode); Pool partition_all_reduce gives Z on all
partitions, DVE takes 1/Z and scales — the PE does no normalization work.
Heads are processed in pairs so exp/recip/band ops cover two heads per
instruction, and the narrow tail j-tiles (8..15) are packed two per PSUM
tile. The output projection is split: pass A (contraction k<8, or less at
earlier pairs) is injected one matmul at a time into phase-2's idle PE
slots and staged in SBUF (yA); phase 3 finishes with pass B + bias (bias
folded in as a rank-1 matmul) and a single DVE add.

Non-causal masks fall back to the general f32r kernel (build_general).
"""

import sys

sys.path.insert(0, "/opt/trn_rl_repo")

import numpy as np

import concourse.bacc as bacc
import concourse.bass as bass
import concourse.bass_isa as bass_isa
import concourse.mybir as mybir
import concourse.tile as tile
from concourse.bass_utils import run_bass_kernel_spmd

F32 = mybir.dt.float32
F32R = mybir.dt.float32r
BF16 = mybir.dt.bfloat16

B, SQ, SK, H, NH = 2, 2048, 2048, 2048, 16
HD = H // NH  # 128
ROWS = 512            # query rows per core
NCORES = 8
KT = H // 128         # 16 contraction tiles for the projections
JT = SK // 128        # 16 key tiles
IT = ROWS // 128      # 4 query 128-tiles per core
SCALE = 1.0 / float(np.sqrt(HD))
NEG = -1e9
MULT = mybir.AluOpType.mult
ADD = mybir.AluOpType.add
EXP = mybir.ActivationFunctionType.Exp
IDENT = mybir.ActivationFunctionType.Identity


def build_causal(cfg=None):
    """Causal-mask kernel, bf16 matmuls, strided query sharding."""
    cfg = {**dict(kvb=4, ppb=10, scb=2, opb=4, rbb=3, yb=3, wqb=2,
                  norm_defer=1, pd=5, pump=0), **(cfg or {})}
    nc = bacc.Bacc()

    xT = nc.dram_tensor("xT", [H, ROWS], BF16, kind="ExternalInput")
    wqT = nc.dram_tensor("wqT", [H, H], BF16, kind="ExternalInput")
    bq = nc.dram_tensor("bq", [H, 1], F32, kind="ExternalInput")
    key = nc.dram_tensor("key", [NH, HD, SK], BF16, kind="ExternalInput")
    vR = nc.dram_tensor("vR", [NH, 128, SK], BF16, kind="ExternalInput")
    band = nc.dram_tensor("band", [128, 64], BF16, kind="ExternalInput")
    bandN = nc.dram_tensor("bandN", [128, 256], BF16, kind="ExternalInput")
    wpT = nc.dram_tensor("wpT", [H, H], BF16, kind="ExternalInput")
    bp1 = nc.dram_tensor("bp1", [1, H], BF16, kind="ExternalInput")
    Y = nc.dram_tensor("Y", [ROWS, H], F32, kind="ExternalOutput")

    with tile.TileContext(nc) as tc:
        with tc.tile_pool(name="res", bufs=1) as res:
            qT_all = res.tile([128, KT, ROWS], BF16)
            attnT_all = res.tile([128, NH, ROWS], BF16)
            # pass-A staging for the 8 output-proj chunks computed during
            # phase 2 (k 0..7 partial sums, flushed from PSUM)
            yA = res.tile([128, 8, 512], F32)
            bq_all = res.tile([128, KT, 1], F32)
            band_sb = res.tile([128, 2, 32], BF16)
            bandN_sb = res.tile([128, 2, 2, 64], BF16)
            bp1_sb = res.tile([1, H], BF16)
            ones_sb = res.tile([128, 1], BF16)
            nc.vector.memset(ones_sb, 1.0)
            ones1_sb = res.tile([1, 128], BF16)
            nc.vector.memset(ones1_sb, 1.0)

            def load_consts():
                nc.sync.dma_start(bq_all,
                                  bq[:, :].rearrange("(t p) x -> p t x", p=128))
                nc.sync.dma_start(band_sb,
                                  band[:, :].rearrange("p (u c) -> p u c", c=32))
                nc.sync.dma_start(bandN_sb, bandN[:, :].rearrange(
                    "p (u j c) -> p u j c", j=2, c=64))
                nc.sync.dma_start(bp1_sb, bp1[:, :])

            # PSUM: sc 2bufs x 2banks + op 3 + zp 1 = 8 banks
            ps = tc.alloc_tile_pool(name="ps", bufs=1, space="PSUM")
            # long-lived SBUF pools
            kv = tc.alloc_tile_pool(name="kv", bufs=cfg["kvb"])
            pp = tc.alloc_tile_pool(name="pp", bufs=cfg["ppb"])
            sS = tc.alloc_tile_pool(name="sS", bufs=2)
            rr = tc.alloc_tile_pool(name="rr", bufs=2)
            ypo = tc.alloc_tile_pool(name="ypo", bufs=cfg["yb"])

            def kv_load(h):
                k_sb = kv.tile([128, SK], BF16, tag="k", name=f"k{h}")
                nc.sync.dma_start(k_sb, key[h, :, :])
                v_sb = kv.tile([128, SK], BF16, tag="v", name=f"v{h}")
                nc.sync.dma_start(v_sb, vR[h, :, :])
                return k_sb, v_sb

            # output-projection weights, split by contraction half:
            # wpA = k 0..7 (used by pass-A chunks injected into phase 2),
            # wpB = k 8..15 (pass B / full chunks in phase 3)
            wpAs, wpBs = {}, {}

            def wp_load(oc, half):
                tag = "wpA" if half == 0 else "wpB"
                wp_sb = wpp.tile([128, 8, 512], BF16, tag=tag,
                                 name=f"{tag}{oc}")
                wp_ap = wpT[1024 * half:1024 * (half + 1),
                            512 * oc:512 * (oc + 1)].rearrange(
                    "(k p) o -> p k o", p=128)
                nc.sync.dma_start(wp_sb, wp_ap)
                return wp_sb

            # ---- phase 2: attention, head pairs g -> heads (2g, 2g+1) ----
            # Z = colsum(p) via Pool partition_all_reduce on S, then
            # rb = 1/Z (DVE) and attnT = op * rb (DVE). No PE involvement.
            norm_pend = []

            def do_norm(g, S, ops):
                Zb = rr.tile([128, 2, 512], F32, tag="Zb", bufs=cfg["rbb"],
                             name=f"Zb{g}")
                nc.gpsimd.partition_all_reduce(Zb, S, 128, bass_isa.ReduceOp.add)
                rb = rr.tile([128, 2, 512], F32, tag="rb", bufs=cfg["rbb"],
                             name=f"rb{g}")
                nc.vector.reciprocal(rb, Zb)
                for u in range(2):
                    h = 2 * g + u
                    nc.vector.tensor_tensor(attnT_all[:, h, :], ops[u],
                                            rb[:, u, :], op=MULT)

            chunk_state = {}
            chunk_k0 = {}  # c -> first k left for pass B

            def inject_mm(c, k, klast):
                # one matmul of pass-A chunk c (psy = sum_{k<=klast} ...),
                # spread across phase-2 steps to fit the fragmented PE idle
                oc, it = c // 4, c % 4
                if c not in chunk_state:
                    chunk_state[c] = ps.tile([128, 512], F32, tag="op",
                                             bufs=cfg["opb"], name=f"pyA{c}")
                psy = chunk_state[c]
                nc.tensor.matmul(psy, attnT_all[:, k, 128 * it:128 * (it + 1)],
                                 wpAs[oc][:, k, :], start=(k == 0),
                                 stop=(k == klast), skip_group_check=True)
                if k == klast:
                    nc.vector.tensor_scalar(yA[:, c, :], psy, 1.0, None,
                                            op0=MULT)
                    del chunk_state[c]
                    chunk_k0[c] = klast + 1

            def pair_gen(g):
                h0, h1 = 2 * g, 2 * g + 1
                if h0 not in kvs:
                    kvs[h0] = kv_load(h0)
                if h1 not in kvs:
                    kvs[h1] = kv_load(h1)
                if 2 * (g + 1) not in kvs and g + 1 < 8:  # prefetch next pair
                    kvs[2 * (g + 1)] = kv_load(2 * (g + 1))
                    kvs[2 * (g + 1) + 1] = kv_load(2 * (g + 1) + 1)
                if g == 1:
                    wpAs[0] = wp_load(0, 0)
                elif g == 3:
                    wpAs[1] = wp_load(1, 0)
                elif g == 6:
                    wpBs[0] = wp_load(0, 1)
                elif g == 7:
                    wpBs[1] = wp_load(1, 1)
                k0, v0 = kvs.pop(h0)
                k1, v1 = kvs.pop(h1)
                ks, vs = (k0, k1), (v0, v1)

                S = sS.tile([128, 2, 512], BF16, tag="S", name=f"S{g}")
                op0 = ps.tile([128, 512], F32, tag="op", bufs=cfg["opb"],
                              name=f"op{h0}")
                op1 = ps.tile([128, 512], F32, tag="op", bufs=cfg["opb"],
                              name=f"op{h1}")
                ops = (op0, op1)
                pend = []  # consume-callbacks, one step late

                def consume_wide(jt, p_sb):
                    b0 = 32 * jt
                    W = 512 - b0
                    for u in range(2):
                        nc.tensor.matmul(ops[u][:, b0:], vs[u][:, 128 * jt:128 * (jt + 1)],
                                         p_sb[:, u, :W],
                                         start=(jt == 0), stop=False,
                                         skip_group_check=True)
                    if jt == 0:
                        nc.vector.tensor_scalar(S, p_sb, 1.0, None, op0=MULT)
                    else:
                        nc.vector.tensor_tensor(S[:, :, b0:], S[:, :, b0:],
                                                p_sb[:, :, :W], op=ADD)

                def consume_narrow(jt0, W0, p_sb):
                    b0 = 32 * jt0
                    for u in range(2):
                        for jj in range(2):
                            nc.tensor.matmul(
                                ops[u][:, b0:b0 + W0],
                                vs[u][:, 128 * (jt0 + jj):128 * (jt0 + jj + 1)],
                                p_sb[:, u, jj, :W0],
                                start=False, stop=(jt0 + jj == JT - 1),
                                skip_group_check=True)
                    for jj in range(2):
                        nc.vector.tensor_tensor(S[:, :, b0:b0 + W0],
                                                S[:, :, b0:b0 + W0],
                                                p_sb[:, :, jj, :W0], op=ADD)

                # wide steps: one j-tile each (jt 0..7); narrow steps: two
                # j-tiles share one 2-bank PSUM tile (jt 8..15)
                for st in range(12):
                    if st < 8:
                        jt = st
                        b0 = 32 * jt
                        W = 512 - b0
                        sc = ps.tile([128, 2, 512], F32, tag="sc", bufs=cfg["scb"],
                                     name=f"sc{g}_{st}")
                        for u in range(2):
                            nc.tensor.matmul(sc[:, u, b0:],
                                             ks[u][:, 128 * jt:128 * (jt + 1)],
                                             qT_all[:, 2 * g + u, b0:],
                                             start=True, stop=True)
                        p_sb = pp.tile([128, 2, 512], BF16, tag="p",
                                       bufs=cfg["ppb"], name=f"p{g}_{st}")
                        nc.scalar.activation(p_sb[:, :, :W], sc[:, :, b0:], EXP,
                                             scale=SCALE)
                        nc.gpsimd.tensor_tensor(p_sb[:, :, 0:32], p_sb[:, :, 0:32],
                                                band_sb, op=MULT)
                        pend.append((consume_wide, (jt, p_sb)))
                    else:
                        jt0 = 8 + 2 * (st - 8)
                        b0 = 32 * jt0
                        W0 = 512 - b0  # both sub-tiles computed at width W0
                        sc = ps.tile([128, 2, 2, 256], F32, tag="sc",
                                     bufs=cfg["scb"], name=f"sc{g}_{st}")
                        for u in range(2):
                            for jj in range(2):
                                nc.tensor.matmul(
                                    sc[:, u, jj, :W0],
                                    ks[u][:, 128 * (jt0 + jj):128 * (jt0 + jj + 1)],
                                    qT_all[:, 2 * g + u, b0:],
                                    start=True, stop=True)
                        p_sb = pp.tile([128, 2, 2, 256], BF16, tag="p",
                                       bufs=cfg["ppb"], name=f"p{g}_{st}")
                        nc.scalar.activation(p_sb[:, :, :, :W0], sc[:, :, :, :W0],
                                             EXP, scale=SCALE)
                        nc.vector.tensor_tensor(p_sb[:, :, :, 0:64],
                                                p_sb[:, :, :, 0:64],
                                                bandN_sb, op=MULT)
                        pend.append((consume_narrow, (jt0, W0, p_sb)))
                    if len(pend) > cfg["pd"]:
                        fn, args = pend.pop(0)
                        fn(*args)
                    if st == cfg["norm_defer"] and norm_pend:
                        do_norm(*norm_pend.pop(0))
                    if g in (2, 3):
                        # two chunks, k limited by normalized heads
                        klast = 3 if g == 2 else 5
                        if 2 <= st <= 2 + klast:
                            inject_mm((g - 2) * 2, st - 2, klast)
                        if 4 <= st <= 4 + klast:
                            inject_mm((g - 2) * 2 + 1, st - 4, klast)
                    elif g >= 4:
                        if 1 <= st <= 8:
                            inject_mm(g, st - 1, 7)
                    yield
                while pend:
                    fn, args = pend.pop(0)
                    fn(*args)
                norm_pend.append((g, S, ops))

            kvs = {}
            gen0 = pair_gen(0)

            # ---- phase 1: q projection (o-chunks of 512, t = o-tile) ----
            with tc.tile_pool(name="p1", bufs=1) as p1, \
                 tc.tile_pool(name="p1w", bufs=cfg["wqb"]) as p1w:
                xT_all = p1.tile([128, KT, ROWS], BF16)
                xT_ap = xT[:, :].rearrange("(t p) i -> p t i", p=128)
                for c in range(4):
                    wq_sb = p1w.tile([128, KT, 512], BF16, tag="wq")
                    wq_ap = wqT[:, 512 * c:512 * (c + 1)].rearrange(
                        "(k p) o -> p k o", p=128)
                    if c == 0:
                        # interleave xT / wq-c0 pieces: first matmuls unblock
                        # after ~0.5MB instead of ~4MB
                        for q0, q1 in [(0, 1), (1, 2), (2, 4), (4, 8),
                                       (8, 12), (12, 16)]:
                            nc.sync.dma_start(xT_all[:, q0:q1, :],
                                              xT_ap[:, q0:q1, :])
                            nc.sync.dma_start(wq_sb[:, q0:q1, :],
                                              wq_ap[:, q0:q1, :])
                        load_consts()
                    else:
                        nc.sync.dma_start(wq_sb, wq_ap)
                    if c == 2:
                        for h in range(4):  # prefetch k/v for pairs 0-1
                            kvs[h] = kv_load(h)
                    for u in range(4):
                        t = 4 * c + u
                        psq = ps.tile([128, 512], F32, tag="op", bufs=cfg["opb"],
                                      name=f"psq{t}")
                        for k in range(KT):
                            nc.tensor.matmul(psq, wq_sb[:, k, 128 * u:128 * (u + 1)],
                                             xT_all[:, k, :],
                                             start=(k == 0), stop=(k == KT - 1))
                        if c == 3:
                            # last chunk's bias on DVE: frees ACT so pair-0
                            # exps start sooner at the phase transition
                            nc.vector.tensor_scalar(qT_all[:, t, :], psq,
                                                    bq_all[:, t, :], None,
                                                    op0=ADD)
                        else:
                            nc.scalar.activation(qT_all[:, t, :], psq, IDENT,
                                                 bias=bq_all[:, t, :])
                        if c >= 1 and cfg["pump"]:
                            # pump one attention step of pair 0 between
                            # q-projection tiles: its exp/Z work hides under
                            # phase-1 PE time
                            next(gen0, None)

            wpp = tc.alloc_tile_pool(name="wpp", bufs=2)
            for _ in gen0:
                pass
            for g in range(1, 8):
                for _ in pair_gen(g):
                    pass
            while norm_pend:
                do_norm(*norm_pend.pop(0))

            # ---- phase 3 ----
            # pass B for oc 0,1 (k 8..15 + bias, added to the staged yA),
            # then full chunks for oc 2,3
            wpAs[2] = wp_load(2, 0)
            wpBs[2] = wp_load(2, 1)
            # oc 0: run all four chunks' k<14 first (independent of the
            # last pair's norm chain), then finish k14/15 + bias per chunk.
            # Two chunks borrow idle sc-tag PSUM slots to avoid aliasing
            # pair-7's still-live op accumulators.
            psysB = []
            for it in range(IT):
                c = it
                k0c = chunk_k0[c]
                psy = ps.tile([128, 512], F32,
                              tag=("sc" if it < 2 else "op"),
                              bufs=(cfg["scb"] if it < 2 else cfg["opb"]),
                              name=f"psyB0_{it}")
                for k in range(k0c, 14):
                    wsrc = (wpAs[0][:, k, :] if k < 8
                            else wpBs[0][:, k - 8, :])
                    nc.tensor.matmul(psy, attnT_all[:, k, 128 * it:128 * (it + 1)],
                                     wsrc, start=(k == k0c), stop=False,
                                     skip_group_check=True)
                psysB.append((it, c, psy))
            for it, c, psy in psysB:
                for k in (14, 15):
                    nc.tensor.matmul(psy, attnT_all[:, k, 128 * it:128 * (it + 1)],
                                     wpBs[0][:, k - 8, :], start=False,
                                     stop=False, skip_group_check=True)
                nc.tensor.matmul(psy, ones1_sb, bp1_sb[:, 0:512],
                                 start=False, stop=True, skip_group_check=True)
                y_sb = ypo.tile([128, 512], F32, tag="y", name=f"yB0_{it}")
                nc.vector.tensor_tensor(y_sb, psy, yA[:, c, :], op=ADD)
                nc.sync.dma_start(Y[128 * it:128 * (it + 1), 0:512], y_sb)
            wpAs[3] = wp_load(3, 0)
            wpBs[3] = wp_load(3, 1)
            for oc in range(1, 2):
                for it in range(IT):
                    c = 4 * oc + it
                    k0c = chunk_k0[c]
                    psy = ps.tile([128, 512], F32, tag="op", bufs=cfg["opb"],
                                  name=f"psyB{oc}_{it}")
                    for k in range(k0c, KT):
                        wsrc = (wpAs[oc][:, k, :] if k < 8
                                else wpBs[oc][:, k - 8, :])
                        nc.tensor.matmul(psy, attnT_all[:, k, 128 * it:128 * (it + 1)],
                                         wsrc,
                                         start=(k == k0c), stop=False,
                                         skip_group_check=True)
                    nc.tensor.matmul(psy, ones1_sb,
                                     bp1_sb[:, 512 * oc:512 * (oc + 1)],
                                     start=False, stop=True, skip_group_check=True)
                    y_sb = ypo.tile([128, 512], F32, tag="y", name=f"y{oc}_{it}")
                    nc.vector.tensor_tensor(y_sb, psy, yA[:, c, :], op=ADD)
                    nc.sync.dma_start(
                        Y[128 * it:128 * (it + 1), 512 * oc:512 * (oc + 1)], y_sb)
            for oc in range(2, 4):
                for it in range(IT):
                    psy = ps.tile([128, 512], F32, tag="op", bufs=cfg["opb"],
                                  name=f"psy{oc}_{it}")
                    for k in range(KT):
                        wp_sb = wpAs[oc] if k < 8 else wpBs[oc]
                        nc.tensor.matmul(psy, attnT_all[:, k, 128 * it:128 * (it + 1)],
                                         wp_sb[:, k % 8, :],
                                         start=(k == 0), stop=False,
                                         skip_group_check=True)
                    nc.tensor.matmul(psy, ones1_sb,
                                     bp1_sb[:, 512 * oc:512 * (oc + 1)],
                                     start=False, stop=True, skip_group_check=True)
                    y_sb = ypo.tile([128, 512], F32, tag="y", name=f"y{oc}_{it}")
                    nc.scalar.copy(y_sb, psy)
                    nc.sync.dma_start(
                        Y[128 * it:128 * (it + 1), 512 * oc:512 * (oc + 1)], y_sb)
            wpp.release()
            ypo.release()
            rr.release()
            sS.release()
            pp.release()
            kv.release()
            ps.release()

    nc.compile()
    return nc


def build_general(mm_dt=F32R, mask_dt=F32):
    """General-mask fallback (row-block sharding, f32r matmuls). This is the
    previous-generation kernel, kept for non-causal attention_mask inputs."""
    cfg = dict(kv=2, tp=4, pp=4, p1w=2, scb=4, zpb=1, opb=1, GS=1, wpp=4)
    GS = cfg["GS"]
    nc = bacc.Bacc()

    xT = nc.dram_tensor("xT", [H, ROWS], mm_dt, kind="ExternalInput")
    wqT = nc.dram_tensor("wqT", [H, H], mm_dt, kind="ExternalInput")
    bq = nc.dram_tensor("bq", [H, 1], F32, kind="ExternalInput")
    key = nc.dram_tensor("key", [NH, HD, SK], mm_dt, kind="ExternalInput")
    value = nc.dram_tensor("value", [NH, SK, HD], mm_dt, kind="ExternalInput")
    maskT = nc.dram_tensor("maskT", [SK, ROWS], mask_dt, kind="ExternalInput")
    wpT = nc.dram_tensor("wpT", [H, H], mm_dt, kind="ExternalInput")
    bpB = nc.dram_tensor("bpB", [128, H], F32, kind="ExternalInput")
    onesd = nc.dram_tensor("onesd", [128, 1], mm_dt, kind="ExternalInput")
    ones1d = nc.dram_tensor("ones1d", [1, 128], mm_dt, kind="ExternalInput")
    Y = nc.dram_tensor("Y", [ROWS, H], F32, kind="ExternalOutput")

    with tile.TileContext(nc) as tc:
        with tc.tile_pool(name="res", bufs=1) as res:
            qT_all = res.tile([128, KT, ROWS], mm_dt)
            attnT_all = res.tile([128, NH, ROWS], mm_dt)
            maskT_all = res.tile([128, JT, ROWS], mask_dt)
            bq_all = res.tile([128, KT, 1], F32)
            nc.sync.dma_start(bq_all, bq[:, :].rearrange("(t p) x -> p t x", p=128))
            bpB_all = res.tile([128, H], F32)
            nc.sync.dma_start(bpB_all, bpB[:, :])
            ones_sb = res.tile([128, 1], mm_dt)
            nc.sync.dma_start(ones_sb, onesd[:, :])
            ones1_sb = res.tile([1, 128], mm_dt)
            nc.sync.dma_start(ones1_sb, ones1d[:, :])

            wpp = tc.alloc_tile_pool(name="wpp", bufs=cfg["wpp"])
            kv = tc.alloc_tile_pool(name="kv", bufs=cfg["kv"])
            tp = tc.alloc_tile_pool(name="tp", bufs=cfg["tp"])
            pp = tc.alloc_tile_pool(name="pp", bufs=cfg["pp"])
            ps_s = tc.alloc_tile_pool(name="ps_s", bufs=cfg["scb"], space="PSUM")
            ps_z = tc.alloc_tile_pool(name="ps_z", bufs=cfg["zpb"], space="PSUM")
            ps_o = tc.alloc_tile_pool(name="ps_o", bufs=cfg["opb"], space="PSUM")

            with tc.tile_pool(name="p1", bufs=1) as p1, \
                 tc.tile_pool(name="p1w", bufs=cfg["p1w"]) as p1w, \
                 tc.tile_pool(name="ps_q", bufs=2, space="PSUM") as ps_q:
                xT_all = p1.tile([128, KT, ROWS], mm_dt)
                xT_ap = xT[:, :].rearrange("(t p) i -> p t i", p=128)
                for k in range(KT):
                    nc.sync.dma_start(xT_all[:, k, :], xT_ap[:, k, :])
                wqT_ap = wqT[:, :].rearrange("(a p) o -> p a o", p=128)
                for t in range(KT):
                    w_sb = p1w.tile([128, KT, 128], mm_dt, tag="wq")
                    nc.sync.dma_start(w_sb[:, :KT // 2, :],
                                      wqT_ap[:, :KT // 2, 128 * t:128 * (t + 1)])
                    nc.sync.dma_start(w_sb[:, KT // 2:, :],
                                      wqT_ap[:, KT // 2:, 128 * t:128 * (t + 1)])
                    psq = ps_q.tile([128, ROWS], F32, tag="psq")
                    for k in range(KT):
                        nc.tensor.matmul(psq, w_sb[:, k, :], xT_all[:, k, :],
                                         start=(k == 0), stop=(k == KT - 1))
                    nc.scalar.activation(qT_all[:, t, :], psq, IDENT,
                                         bias=bq_all[:, t, :])

            sm = tc.alloc_tile_pool(name="sm", bufs=2)
            maskT_ap = maskT[:, :].rearrange("(t p) i -> p t i", p=128)
            for j in range(JT):
                nc.sync.dma_start(maskT_all[:, j, :], maskT_ap[:, j, :])
            for h in range(NH):
                k_sbs, v_sbs = [], []
                for hf in range(2):
                    k_sb = kv.tile([128, JT // 2, 128], mm_dt, tag="k",
                                   name=f"k{h}_{hf}")
                    nc.sync.dma_start(
                        k_sb, key[h, :, 1024 * hf:1024 * (hf + 1)]
                        .rearrange("d (a j) -> d a j", j=128))
                    v_sb = kv.tile([128, JT // 2, 128], mm_dt, tag="v",
                                   name=f"v{h}_{hf}")
                    nc.sync.dma_start(
                        v_sb, value[h, 1024 * hf:1024 * (hf + 1), :]
                        .rearrange("(a p) d -> p a d", p=128))
                    k_sbs.append(k_sb)
                    v_sbs.append(v_sb)

                zp = ps_z.tile([1, ROWS], F32, tag="z")
                op = ps_o.tile([128, ROWS], F32, tag="o")
                pend = []

                def consume(gp, p_tile):
                    for uu in range(p_tile.shape[1]):
                        jtc = GS * gp + uu
                        nc.tensor.matmul(op, v_sbs[jtc // 8][:, jtc % 8, :],
                                         p_tile[:, uu, :],
                                         start=(jtc == 0), stop=(jtc == JT - 1))
                        nc.tensor.matmul(zp, ones_sb, p_tile[:, uu, :],
                                         start=(jtc == 0), stop=(jtc == JT - 1))

                for gidx in range(JT // GS):
                    W = ROWS
                    sc = ps_s.tile([128, GS * W], F32, tag="s", name=f"sc{h}_{gidx}")
                    t_sb = tp.tile([128, GS, W], F32, tag="t", name=f"t{h}_{gidx}")
                    for u in range(GS):
                        jt = GS * gidx + u
                        nc.tensor.matmul(sc[:, W * u:W * (u + 1)],
                                         k_sbs[jt // 8][:, jt % 8, :],
                                         qT_all[:, h, :], start=True, stop=True)
                        nc.vector.scalar_tensor_tensor(
                            t_sb[:, u, :], sc[:, W * u:W * (u + 1)],
                            1.0, maskT_all[:, jt, :], MULT, ADD)
                    p_sb = pp.tile([128, GS, W], mm_dt, tag="p", name=f"p{h}_{gidx}")
                    nc.scalar.activation(p_sb, t_sb, EXP, scale=SCALE)
                    pend.append((gidx, p_sb))
                    if len(pend) > 1:
                        consume(*pend.pop(0))
                while pend:
                    consume(*pend.pop(0))

                rc = sm.tile([1, ROWS], mm_dt, tag="rc")
                with nc.allow_low_precision(reason="f32r reciprocal storage"):
                    nc.vector.reciprocal(rc, zp)
                bc = ps_s.tile([128, ROWS], F32, tag="s")
                nc.tensor.matmul(bc, ones1_sb, rc, start=True, stop=True)
                rb = sm.tile([128, ROWS], F32, tag="rb")
                nc.scalar.copy(rb, bc)
                nc.vector.tensor_tensor(attnT_all[:, h, :], op, rb, op=MULT)

            sm.release()
            ps_o.release()
            ps_z.release()
            ps_s.release()
            pp.release()
            tp.release()
            kv.release()

            with tc.tile_pool(name="ypo", bufs=2) as ypo, \
                 tc.tile_pool(name="ps_y", bufs=4, space="PSUM") as ps_y:
                wpT_ap = wpT[:, :].rearrange("(a p) o -> p a o", p=128)
                for half in range(2):
                    o0 = 1024 * half
                    psys = []
                    for it in range(IT):
                        psy = ps_y.tile([128, 1024], F32, tag="y",
                                        name=f"psy{half}_{it}")
                        psys.append(psy)
                    for k in range(KT):
                        wp_sb = wpp.tile([128, 1024], mm_dt, tag="wp")
                        nc.sync.dma_start(wp_sb, wpT_ap[:, k, o0:o0 + 1024])
                        for it in range(IT):
                            att = attnT_all[:, k, 128 * it:128 * (it + 1)]
                            for nb in range(2):
                                nc.tensor.matmul(
                                    psys[it][:, 512 * nb:512 * (nb + 1)],
                                    att, wp_sb[:, 512 * nb:512 * (nb + 1)],
                                    start=(k == 0), stop=(k == KT - 1))
                    for it in range(IT):
                        y_sb = ypo.tile([128, 1024], F32, tag="ysb")
                        nc.vector.tensor_tensor(y_sb, psys[it],
                                                bpB_all[:, o0:o0 + 1024], op=ADD)
                        nc.sync.dma_start(
                            Y[128 * it:128 * (it + 1), o0:o0 + 1024], y_sb)
            wpp.release()

    nc.compile()
    return nc


_CACHE = {}


def _get_nc(mm_dt=None, mask_dt=None, causal=True, cfg=None):
    """test.py compatibility: causal=True returns the bf16 causal kernel
    (mm_dt/mask_dt ignored); causal=False returns the general kernel."""
    if causal:
        ck = ("causal", tuple(sorted((cfg or {}).items())))
        if ck not in _CACHE:
            _CACHE[ck] = build_causal(cfg)
        return _CACHE[ck]
    ck = ("gen", str(mm_dt), str(mask_dt))
    if ck not in _CACHE:
        _CACHE[ck] = build_general(mm_dt if mm_dt is not None else F32R,
                                   mask_dt if mask_dt is not None else F32)
    return _CACHE[ck]


def _is_causal(attention_mask):
    """True if the mask is exactly the standard causal additive mask."""
    m = attention_mask
    if m.shape != (B, 1, SQ, SK):
        return False
    m0 = np.asarray(m[0, 0])
    tri = np.tril(np.ones((SQ, SK), dtype=bool))
    ref = np.where(tri, np.float32(0.0), np.float32(NEG))
    if not np.array_equal(m0, ref):
        return False
    for b in range(1, B):
        if not np.array_equal(np.asarray(m[b, 0]), m0):
            return False
    return True


def _kernel_causal(hidden_states, key, value, w_q, b_q, w_proj, b_proj,
                   _cfg=None, _trace=False):
    import ml_dtypes
    bf16 = ml_dtypes.bfloat16
    nc = _get_nc(causal=True, cfg=_cfg)

    wqT = np.ascontiguousarray(w_q.T).astype(bf16)
    wpT = np.ascontiguousarray(w_proj.T).astype(bf16)
    bq2 = np.ascontiguousarray(b_q[:, None]).astype(np.float32)
    bp1 = np.ascontiguousarray(b_proj[None, :]).astype(bf16)
    key_b = [np.ascontiguousarray(key[b * NH:(b + 1) * NH]).astype(bf16)
             for b in range(B)]
    # vR[h, p, a*HD + d] = v[h, a*128 + p, d]  (4KB contiguous runs per line)
    vR_b = [np.ascontiguousarray(
        value[b].reshape(NH, SK // 128, 128, HD).transpose(0, 2, 1, 3)
        .reshape(NH, 128, SK)).astype(bf16) for b in range(B)]
    inv_scale = np.float32(1.0 / SCALE)

    # band mask: for key-tile jt, col u of the 32-col band (query row
    # 128*jt + 4*u + s, key 128*jt + p): unmasked iff p <= 4*u + s
    p_idx = np.arange(128)[:, None]
    u_idx = np.arange(32)[None, :]

    in_maps = []
    for c in range(NCORES):
        b, s = c // 4, c % 4
        rows = s + 4 * np.arange(ROWS)
        xT_c = np.ascontiguousarray(hidden_states[b, rows, :].T).astype(bf16)
        # multiplicative post-exp masks: 1 keep, 0 drop
        band_c = (p_idx <= 4 * u_idx + s).astype(np.float32)
        band2 = np.ascontiguousarray(
            np.concatenate([band_c, band_c], axis=1)).astype(bf16)
        # bandN [128, (u, jj, 64)]: jj=0 -> [band | 1]; jj=1 -> [0 | band]
        one = np.ones((128, 32), np.float32)
        zer = np.zeros((128, 32), np.float32)
        row = np.concatenate([band_c, one, zer, band_c], axis=1)  # [128, 128]
        bandN_c = np.ascontiguousarray(
            np.concatenate([row, row], axis=1)).astype(bf16)  # [128, 256]
        in_maps.append(dict(
            xT=xT_c, wqT=wqT, bq=bq2, key=key_b[b], vR=vR_b[b],
            band=band2, bandN=bandN_c, wpT=wpT, bp1=bp1,
        ))

    kw = {}
    if _trace:
        kw = dict(trace=True, trace_cores=list(range(NCORES)), stitch_traces=False)
    res = run_bass_kernel_spmd(nc, in_maps, core_ids=list(range(NCORES)), **kw)
    if _trace:
        kernel._last_result = res

    out = np.empty((B, SQ, H), dtype=np.float32)
    for c in range(NCORES):
        b, s = c // 4, c % 4
        rows = s + 4 * np.arange(ROWS)
        out[b, rows, :] = res.results[c]["Y"]
    return out


def _kernel_general(hidden_states, key, value, attention_mask,
                    w_q, b_q, w_proj, b_proj, _mm_dt=F32R):
    nc = _get_nc(_mm_dt, F32, causal=False)
    wqT = np.ascontiguousarray(w_q.T)
    wpT = np.ascontiguousarray(w_proj.T)
    bq2 = np.ascontiguousarray(b_q[:, None]).astype(np.float32)
    bpB = np.ascontiguousarray(
        np.broadcast_to(b_proj[None, :], (128, H))).astype(np.float32)
    key_b = [np.ascontiguousarray(key[b * NH:(b + 1) * NH]) for b in range(B)]
    val_b = [np.ascontiguousarray(value[b]) for b in range(B)]
    inv_scale = np.float32(1.0 / SCALE)

    in_maps = []
    for c in range(NCORES):
        b, sидx = c // 4, c % 4
        rows = np.arange(ROWS * sидx, ROWS * sидx + ROWS)
        xT_c = np.ascontiguousarray(hidden_states[b, rows, :].T)
        maskT_c = np.ascontiguousarray(
            (attention_mask[b, 0, rows, :].T * inv_scale).astype(np.float32))
        in_maps.append(dict(
            xT=xT_c, wqT=wqT, bq=bq2, key=key_b[b], value=val_b[b],
            maskT=maskT_c, wpT=wpT, bpB=bpB,
            onesd=np.ones((128, 1), dtype=np.float32),
            ones1d=np.ones((1, 128), dtype=np.float32),
        ))
    res = run_bass_kernel_spmd(nc, in_maps, core_ids=list(range(NCORES)))
    out = np.empty((B, SQ, H), dtype=np.float32)
    for c in range(NCORES):
        b, sидx = c // 4, c % 4
        rows = np.arange(ROWS * sидx, ROWS * sидx + ROWS)
        out[b, rows, :] = res.results[c]["Y"]
    return out


def kernel(hidden_states, key, value, attention_mask, w_q, b_q, w_proj, b_proj,
           _mm_dt=F32R, _trace=False, _cfg=None):
    hidden_states = np.asarray(hidden_states)
    key = np.asarray(key)
    value = np.asarray(value)
    attention_mask = np.asarray(attention_mask)
    w_q = np.asarray(w_q)
    b_q = np.asarray(b_q)
    w_proj = np.asarray(w_proj)
    b_proj = np.asarray(b_proj)

    if _is_causal(attention_mask):
        return _kernel_causal(hidden_states, key, value, w_q, b_q,
                              w_proj, b_proj, _cfg=_cfg, _trace=_trace)
    return _kernel_general(hidden_states, key, value, attention_mask,
                           w_q, b_q, w_proj, b_proj, _mm_dt=_mm_dt)


BF16_ = BF16  # back-compat alias


if __name__ == "__main__":
    pass


# revision 61
# speedup vs baseline: 1.0208x; 1.0141x over previous
"""Trainium2 Bass kernel for nn_CrossLayerAttention_309237645906.

Reference computation (B=2, SQ=SK=2048, H=2048, NH=16, HD=128, fp32):
    q = hidden @ w_q.T + b_q                     -> [B, NH, SQ, HD]
    scores = mask + scale * q @ k                (k given as [B*NH, HD, SK])
    probs = softmax(scores)                      (fp32)
    out = (probs @ v)                            -> [B, SQ, H]
    y = out @ w_proj.T + b_proj

Causal fast path (used when the mask is exactly the standard causal mask):

Sharding: 8 cores = (batch b = c//4) x (strided query set s = c%4: local
query column i <-> global row 4*i + s). The stride-4 mapping makes the
causal work profile IDENTICAL on every core (required: one SPMD program),
and exactly ideal: for key-tile jt only query cols [32*jt, 512) can be
unmasked, so scores/pv matmuls run width W(jt) = 512 - 32*jt
(sum_j W = 4352 = the causal optimum). Only a 32-col "band" at the left
edge of each j-tile straddles the diagonal; a per-core [128, 32]
multiplicative 0/1 mask (same for every jt and head) is applied to the
exp output (exact: p = exp * {0,1}) — on Pool for the wide steps, on DVE
for the narrow tail steps so the pair-end drain never queues behind the
Pool partition_all_reduce.

All matmul operands are bf16 (full PE rate, half DMA bytes); accumulation
is f32 in PSUM; softmax runs in f32/bf16 mixed (~3e-3 max rel err, well
under the 2e-2 gate). Softmax denominator: p-tiles are summed on DVE into
S [128, 2, 512] (bf16 2x mode); Pool partition_all_reduce gives Z on all
partitions, DVE takes 1/Z and scales — the PE does no normalization work.
Heads are processed in pairs so exp/recip/band ops cover two heads per
instruction, and the narrow tail j-tiles (8..15) are packed two per PSUM
tile. The output projection is split: pass A (contraction k<8, or less at
earlier pairs) is injected one matmul at a time into phase-2's idle PE
slots and staged in SBUF (yA); phase 3 finishes with pass B + bias (bias
folded in as a rank-1 matmul) and a single DVE add.

Non-causal masks fall back to the general f32r kernel (build_general).
"""

import sys

sys.path.insert(0, "/opt/trn_rl_repo")

import numpy as np

import concourse.bacc as bacc
import concourse.bass as bass
import concourse.bass_isa as bass_isa
import concourse.mybir as mybir
import concourse.tile as tile
from concourse.bass_utils import run_bass_kernel_spmd

F32 = mybir.dt.float32
F32R = mybir.dt.float32r
BF16 = mybir.dt.bfloat16

B, SQ, SK, H, NH = 2, 2048, 2048, 2048, 16
HD = H // NH  # 128
ROWS = 512            # query rows per core
NCORES = 8
KT = H // 128         # 16 contraction tiles for the projections
JT = SK // 128        # 16 key tiles
IT = ROWS // 128      # 4 query 128-tiles per core
SCALE = 1.0 / float(np.sqrt(HD))
NEG = -1e9
MULT = mybir.AluOpType.mult
ADD = mybir.AluOpType.add
EXP = mybir.ActivationFunctionType.Exp
IDENT = mybir.ActivationFunctionType.Identity


def build_causal(cfg=None):
    """Causal-mask kernel, bf16 matmuls, strided query sharding."""
    cfg = {**dict(kvb=4, ppb=10, scb=2, opb=4, rbb=3, yb=3, wqb=2,
                  norm_defer=1, pd=5, pump=0), **(cfg or {})}
    nc = bacc.Bacc()

    xT = nc.dram_tensor("xT", [H, ROWS], BF16, kind="ExternalInput")
    wqT = nc.dram_tensor("wqT", [H, H], BF16, kind="ExternalInput")
    bq = nc.dram_tensor("bq", [H, 1], F32, kind="ExternalInput")
    key = nc.dram_tensor("key", [NH, HD, SK], BF16, kind="ExternalInput")
    vR = nc.dram_tensor("vR", [NH, 128, SK], BF16, kind="ExternalInput")
    band = nc.dram_tensor("band", [128, 64], BF16, kind="ExternalInput")
    bandN = nc.dram_tensor("bandN", [128, 256], BF16, kind="ExternalInput")
    wpT = nc.dram_tensor("wpT", [H, H], BF16, kind="ExternalInput")
    bp1 = nc.dram_tensor("bp1", [1, H], BF16, kind="ExternalInput")
    Y = nc.dram_tensor("Y", [ROWS, H], F32, kind="ExternalOutput")

    with tile.TileContext(nc) as tc:
        with tc.tile_pool(name="res", bufs=1) as res:
            qT_all = res.tile([128, KT, ROWS], BF16)
            attnT_all = res.tile([128, NH, ROWS], BF16)
            # pass-A staging for the 8 output-proj chunks computed during
            # phase 2 (k 0..7 partial sums, flushed from PSUM)
            yA = res.tile([128, 8, 512], F32)
            bq_all = res.tile([128, KT, 1], F32)
            band_sb = res.tile([128, 2, 32], BF16)
            bandN_sb = res.tile([128, 2, 2, 64], BF16)
            bp1_sb = res.tile([1, H], BF16)
            ones_sb = res.tile([128, 1], BF16)
            nc.vector.memset(ones_sb, 1.0)
            ones1_sb = res.tile([1, 128], BF16)
            nc.vector.memset(ones1_sb, 1.0)

            def load_consts():
                nc.sync.dma_start(bq_all,
                                  bq[:, :].rearrange("(t p) x -> p t x", p=128))
                nc.sync.dma_start(band_sb,
                                  band[:, :].rearrange("p (u c) -> p u c", c=32))
                nc.sync.dma_start(bandN_sb, bandN[:, :].rearrange(
                    "p (u j c) -> p u j c", j=2, c=64))
                nc.sync.dma_start(bp1_sb, bp1[:, :])

            # PSUM: sc 2bufs x 2banks + op 3 + zp 1 = 8 banks
            ps = tc.alloc_tile_pool(name="ps", bufs=1, space="PSUM")
            # long-lived SBUF pools
            kv = tc.alloc_tile_pool(name="kv", bufs=cfg["kvb"])
            pp = tc.alloc_tile_pool(name="pp", bufs=cfg["ppb"])
            sS = tc.alloc_tile_pool(name="sS", bufs=2)
            rr = tc.alloc_tile_pool(name="rr", bufs=2)
            ypo = tc.alloc_tile_pool(name="ypo", bufs=cfg["yb"])

            def kv_load(h):
                k_sb = kv.tile([128, SK], BF16, tag="k", name=f"k{h}")
                nc.sync.dma_start(k_sb, key[h, :, :])
                v_sb = kv.tile([128, SK], BF16, tag="v", name=f"v{h}")
                nc.sync.dma_start(v_sb, vR[h, :, :])
                return k_sb, v_sb

            # output-projection weights, split by contraction half:
            # wpA = k 0..7 (used by pass-A chunks injected into phase 2),
            # wpB = k 8..15 (pass B / full chunks in phase 3)
            wpAs, wpBs = {}, {}

            def wp_load(oc, half):
                tag = "wpA" if half == 0 else "wpB"
                wp_sb = wpp.tile([128, 8, 512], BF16, tag=tag,
                                 name=f"{tag}{oc}")
                wp_ap = wpT[1024 * half:1024 * (half + 1),
                            512 * oc:512 * (oc + 1)].rearrange(
                    "(k p) o -> p k o", p=128)
                nc.sync.dma_start(wp_sb, wp_ap)
                return wp_sb

            # ---- phase 2: attention, head pairs g -> heads (2g, 2g+1) ----
            # Z = colsum(p) via Pool partition_all_reduce on S, then
            # rb = 1/Z (DVE) and attnT = op * rb (DVE). No PE involvement.
            norm_pend = []

            def do_norm(g, S, ops):
                Zb = rr.tile([128, 2, 512], F32, tag="Zb", bufs=cfg["rbb"],
                             name=f"Zb{g}")
                nc.gpsimd.partition_all_reduce(Zb, S, 128, bass_isa.ReduceOp.add)
                rb = rr.tile([128, 2, 512], F32, tag="rb", bufs=cfg["rbb"],
                             name=f"rb{g}")
                nc.vector.reciprocal(rb, Zb)
                for u in range(2):
                    h = 2 * g + u
                    nc.vector.tensor_tensor(attnT_all[:, h, :], ops[u],
                                            rb[:, u, :], op=MULT)

            chunk_state = {}
            chunk_k0 = {}  # c -> first k left for pass B

            def inject_mm(c, k, klast):
                # one matmul of pass-A chunk c (psy = sum_{k<=klast} ...),
                # spread across phase-2 steps to fit the fragmented PE idle
                oc, it = c // 4, c % 4
                if c not in chunk_state:
                    chunk_state[c] = ps.tile([128, 512], F32, tag="op",
                                             bufs=cfg["opb"], name=f"pyA{c}")
                psy = chunk_state[c]
                nc.tensor.matmul(psy, attnT_all[:, k, 128 * it:128 * (it + 1)],
                                 wpAs[oc][:, k, :], start=(k == 0),
                                 stop=(k == klast), skip_group_check=True)
                if k == klast:
                    nc.vector.tensor_scalar(yA[:, c, :], psy, 1.0, None,
                                            op0=MULT)
                    del chunk_state[c]
                    chunk_k0[c] = klast + 1

            def pair_gen(g):
                h0, h1 = 2 * g, 2 * g + 1
                if h0 not in kvs:
                    kvs[h0] = kv_load(h0)
                if h1 not in kvs:
                    kvs[h1] = kv_load(h1)
                if 2 * (g + 1) not in kvs and g + 1 < 8:  # prefetch next pair
                    kvs[2 * (g + 1)] = kv_load(2 * (g + 1))
                    kvs[2 * (g + 1) + 1] = kv_load(2 * (g + 1) + 1)
                if g == 1:
                    wpAs[0] = wp_load(0, 0)
                elif g == 3:
                    wpAs[1] = wp_load(1, 0)
                elif g == 6:
                    wpBs[0] = wp_load(0, 1)
                elif g == 7:
                    wpBs[1] = wp_load(1, 1)
                k0, v0 = kvs.pop(h0)
                k1, v1 = kvs.pop(h1)
                ks, vs = (k0, k1), (v0, v1)

                S = sS.tile([128, 2, 512], BF16, tag="S", name=f"S{g}")
                op0 = ps.tile([128, 512], F32, tag="op", bufs=cfg["opb"],
                              name=f"op{h0}")
                op1 = ps.tile([128, 512], F32, tag="op", bufs=cfg["opb"],
                              name=f"op{h1}")
                ops = (op0, op1)
                pend = []  # consume-callbacks, one step late

                def consume_wide(jt, p_sb):
                    b0 = 32 * jt
                    W = 512 - b0
                    for u in range(2):
                        nc.tensor.matmul(ops[u][:, b0:], vs[u][:, 128 * jt:128 * (jt + 1)],
                                         p_sb[:, u, :W],
                                         start=(jt == 0), stop=False,
                                         skip_group_check=True)
                    if jt == 0:
                        nc.vector.tensor_scalar(S, p_sb, 1.0, None, op0=MULT)
                    else:
                        nc.vector.tensor_tensor(S[:, :, b0:], S[:, :, b0:],
                                                p_sb[:, :, :W], op=ADD)

                def consume_narrow(jt0, W0, p_sb):
                    b0 = 32 * jt0
                    for u in range(2):
                        for jj in range(2):
                            nc.tensor.matmul(
                                ops[u][:, b0:b0 + W0],
                                vs[u][:, 128 * (jt0 + jj):128 * (jt0 + jj + 1)],
                                p_sb[:, u, jj, :W0],
                                start=False, stop=(jt0 + jj == JT - 1),
                                skip_group_check=True)
                    for jj in range(2):
                        nc.vector.tensor_tensor(S[:, :, b0:b0 + W0],
                                                S[:, :, b0:b0 + W0],
                                                p_sb[:, :, jj, :W0], op=ADD)

                # wide steps: one j-tile each (jt 0..7); narrow steps: two
                # j-tiles share one 2-bank PSUM tile (jt 8..15)
                for st in range(12):
                    if st < 8:
                        jt = st
                        b0 = 32 * jt
                        W = 512 - b0
                        sc = ps.tile([128, 2, 512], F32, tag="sc", bufs=cfg["scb"],
                                     name=f"sc{g}_{st}")
                        for u in range(2):
                            nc.tensor.matmul(sc[:, u, b0:],
                                             ks[u][:, 128 * jt:128 * (jt + 1)],
                                             qT_all[:, 2 * g + u, b0:],
                                             start=True, stop=True)
                        p_sb = pp.tile([128, 2, 512], BF16, tag="p",
                                       bufs=cfg["ppb"], name=f"p{g}_{st}")
                        nc.scalar.activation(p_sb[:, :, :W], sc[:, :, b0:], EXP,
                                             scale=SCALE)
                        nc.gpsimd.tensor_tensor(p_sb[:, :, 0:32], p_sb[:, :, 0:32],
                                                band_sb, op=MULT)
                        pend.append((consume_wide, (jt, p_sb)))
                    else:
                        jt0 = 8 + 2 * (st - 8)
                        b0 = 32 * jt0
                        W0 = 512 - b0  # both sub-tiles computed at width W0
                        sc = ps.tile([128, 2, 2, 256], F32, tag="sc",
                                     bufs=cfg["scb"], name=f"sc{g}_{st}")
                        for u in range(2):
                            for jj in range(2):
                                nc.tensor.matmul(
                                    sc[:, u, jj, :W0],
                                    ks[u][:, 128 * (jt0 + jj):128 * (jt0 + jj + 1)],
                                    qT_all[:, 2 * g + u, b0:],
                                    start=True, stop=True)
                        p_sb = pp.tile([128, 2, 2, 256], BF16, tag="p",
                                       bufs=cfg["ppb"], name=f"p{g}_{st}")
                        nc.scalar.activation(p_sb[:, :, :, :W0], sc[:, :, :, :W0],
                                             EXP, scale=SCALE)
                        nc.vector.tensor_tensor(p_sb[:, :, :, 0:64],
                                                p_sb[:, :, :, 0:64],
                                                bandN_sb, op=MULT)
                        pend.append((consume_narrow, (jt0, W0, p_sb)))
                    if len(pend) > cfg["pd"]:
                        fn, args = pend.pop(0)
                        fn(*args)
                    if st == cfg["norm_defer"] and norm_pend:
                        do_norm(*norm_pend.pop(0))
                    if g in (2, 3):
                        # two chunks, k limited by normalized heads
                        klast = 3 if g == 2 else 5
                        if 2 <= st <= 2 + klast:
                            inject_mm((g - 2) * 2, st - 2, klast)
                        if 4 <= st <= 4 + klast:
                            inject_mm((g - 2) * 2 + 1, st - 4, klast)
                    elif g >= 4:
                        if 1 <= st <= 8:
                            inject_mm(g, st - 1, 7)
                    yield
                while pend:
                    fn, args = pend.pop(0)
                    fn(*args)
                norm_pend.append((g, S, ops))

            kvs = {}
            gen0 = pair_gen(0)

            # ---- phase 1: q projection (o-chunks of 512, t = o-tile) ----
            with tc.tile_pool(name="p1", bufs=1) as p1, \
                 tc.tile_pool(name="p1w", bufs=cfg["wqb"]) as p1w:
                xT_all = p1.tile([128, KT, ROWS], BF16)
                xT_ap = xT[:, :].rearrange("(t p) i -> p t i", p=128)
                for c in range(4):
                    wq_sb = p1w.tile([128, KT, 512], BF16, tag="wq")
                    wq_ap = wqT[:, 512 * c:512 * (c + 1)].rearrange(
                        "(k p) o -> p k o", p=128)
                    if c == 0:
                        # interleave xT / wq-c0 pieces: first matmuls unblock
                        # after ~0.5MB instead of ~4MB
                        for q0, q1 in [(0, 1), (1, 2), (2, 4), (4, 8),
                                       (8, 12), (12, 16)]:
                            nc.sync.dma_start(xT_all[:, q0:q1, :],
                                              xT_ap[:, q0:q1, :])
                            nc.sync.dma_start(wq_sb[:, q0:q1, :],
                                              wq_ap[:, q0:q1, :])
                        load_consts()
                    elif c == 1:
                        # split: t4 starts on the first half while the
                        # second half transfers (PE exits the DMA-bound
                        # startup region right as c1 is needed)
                        nc.sync.dma_start(wq_sb[:, 0:8, :], wq_ap[:, 0:8, :])
                        nc.sync.dma_start(wq_sb[:, 8:, :], wq_ap[:, 8:, :])
                    else:
                        nc.sync.dma_start(wq_sb, wq_ap)
                    if c == 2:
                        for h in range(4):  # prefetch k/v for pairs 0-1
                            kvs[h] = kv_load(h)
                    for u in range(4):
                        t = 4 * c + u
                        psq = ps.tile([128, 512], F32, tag="op", bufs=cfg["opb"],
                                      name=f"psq{t}")
                        for k in range(KT):
                            nc.tensor.matmul(psq, wq_sb[:, k, 128 * u:128 * (u + 1)],
                                             xT_all[:, k, :],
                                             start=(k == 0), stop=(k == KT - 1))
                        if c == 3:
                            # last chunk's bias on DVE: frees ACT so pair-0
                            # exps start sooner at the phase transition
                            nc.vector.tensor_scalar(qT_all[:, t, :], psq,
                                                    bq_all[:, t, :], None,
                                                    op0=ADD)
                        else:
                            nc.scalar.activation(qT_all[:, t, :], psq, IDENT,
                                                 bias=bq_all[:, t, :])
                        if c >= 1 and cfg["pump"]:
                            # pump one attention step of pair 0 between
                            # q-projection tiles: its exp/Z work hides under
                            # phase-1 PE time
                            next(gen0, None)

            wpp = tc.alloc_tile_pool(name="wpp", bufs=2)
            for _ in gen0:
                pass
            for g in range(1, 8):
                for _ in pair_gen(g):
                    pass
            while norm_pend:
                do_norm(*norm_pend.pop(0))

            # ---- phase 3 ----
            # pass B for oc 0,1 (k 8..15 + bias, added to the staged yA),
            # then full chunks for oc 2,3
            wpAs[2] = wp_load(2, 0)
            wpBs[2] = wp_load(2, 1)
            # oc 0: run all four chunks' k<14 first (independent of the
            # last pair's norm chain), then finish k14/15 + bias per chunk.
            # Two chunks borrow idle sc-tag PSUM slots to avoid aliasing
            # pair-7's still-live op accumulators.
            psysB = []
            for it in range(IT):
                c = it
                k0c = chunk_k0[c]
                psy = ps.tile([128, 512], F32,
                              tag=("sc" if it < 2 else "op"),
                              bufs=(cfg["scb"] if it < 2 else cfg["opb"]),
                              name=f"psyB0_{it}")
                for k in range(k0c, 14):
                    wsrc = (wpAs[0][:, k, :] if k < 8
                            else wpBs[0][:, k - 8, :])
                    nc.tensor.matmul(psy, attnT_all[:, k, 128 * it:128 * (it + 1)],
                                     wsrc, start=(k == k0c), stop=False,
                                     skip_group_check=True)
                psysB.append((it, c, psy))
            for it, c, psy in psysB:
                for k in (14, 15):
                    nc.tensor.matmul(psy, attnT_all[:, k, 128 * it:128 * (it + 1)],
                                     wpBs[0][:, k - 8, :], start=False,
                                     stop=False, skip_group_check=True)
                nc.tensor.matmul(psy, ones1_sb, bp1_sb[:, 0:512],
                                 start=False, stop=True, skip_group_check=True)
                y_sb = ypo.tile([128, 512], F32, tag="y", name=f"yB0_{it}")
                nc.vector.tensor_tensor(y_sb, psy, yA[:, c, :], op=ADD)
                nc.sync.dma_start(Y[128 * it:128 * (it + 1), 0:512], y_sb)
            wpAs[3] = wp_load(3, 0)
            wpBs[3] = wp_load(3, 1)
            for oc in range(1, 2):
                for it in range(IT):
                    c = 4 * oc + it
                    k0c = chunk_k0[c]
                    psy = ps.tile([128, 512], F32, tag="op", bufs=cfg["opb"],
                                  name=f"psyB{oc}_{it}")
                    for k in range(k0c, KT):
                        wsrc = (wpAs[oc][:, k, :] if k < 8
                                else wpBs[oc][:, k - 8, :])
                        nc.tensor.matmul(psy, attnT_all[:, k, 128 * it:128 * (it + 1)],
                                         wsrc,
                                         start=(k == k0c), stop=False,
                                         skip_group_check=True)
                    nc.tensor.matmul(psy, ones1_sb,
                                     bp1_sb[:, 512 * oc:512 * (oc + 1)],
                                     start=False, stop=True, skip_group_check=True)
                    y_sb = ypo.tile([128, 512], F32, tag="y", name=f"y{oc}_{it}")
                    nc.vector.tensor_tensor(y_sb, psy, yA[:, c, :], op=ADD)
                    nc.sync.dma_start(
                        Y[128 * it:128 * (it + 1), 512 * oc:512 * (oc + 1)], y_sb)
            for oc in range(2, 4):
                for it in range(IT):
                    psy = ps.tile([128, 512], F32, tag="op", bufs=cfg["opb"],
                                  name=f"psy{oc}_{it}")
                    for k in range(KT):
                        wp_sb = wpAs[oc] if k < 8 else wpBs[oc]
                        nc.tensor.matmul(psy, attnT_all[:, k, 128 * it:128 * (it + 1)],
                                         wp_sb[:, k % 8, :],
                                         start=(k == 0), stop=False,
                                         skip_group_check=True)
                    nc.tensor.matmul(psy, ones1_sb,
                                     bp1_sb[:, 512 * oc:512 * (oc + 1)],
                                     start=False, stop=True, skip_group_check=True)
                    y_sb = ypo.tile([128, 512], F32, tag="y", name=f"y{oc}_{it}")
                    nc.scalar.copy(y_sb, psy)
                    nc.sync.dma_start(
                        Y[128 * it:128 * (it + 1), 512 * oc:512 * (oc + 1)], y_sb)
            wpp.release()
            ypo.release()
            rr.release()
            sS.release()
            pp.release()
            kv.release()
            ps.release()

    nc.compile()
    return nc


def build_general(mm_dt=F32R, mask_dt=F32):
    """General-mask fallback (row-block sharding, f32r matmuls). This is the
    previous-generation kernel, kept for non-causal attention_mask inputs."""
    cfg = dict(kv=2, tp=4, pp=4, p1w=2, scb=4, zpb=1, opb=1, GS=1, wpp=4)
    GS = cfg["GS"]
    nc = bacc.Bacc()

    xT = nc.dram_tensor("xT", [H, ROWS], mm_dt, kind="ExternalInput")
    wqT = nc.dram_tensor("wqT", [H, H], mm_dt, kind="ExternalInput")
    bq = nc.dram_tensor("bq", [H, 1], F32, kind="ExternalInput")
    key = nc.dram_tensor("key", [NH, HD, SK], mm_dt, kind="ExternalInput")
    value = nc.dram_tensor("value", [NH, SK, HD], mm_dt, kind="ExternalInput")
    maskT = nc.dram_tensor("maskT", [SK, ROWS], mask_dt, kind="ExternalInput")
    wpT = nc.dram_tensor("wpT", [H, H], mm_dt, kind="ExternalInput")
    bpB = nc.dram_tensor("bpB", [128, H], F32, kind="ExternalInput")
    onesd = nc.dram_tensor("onesd", [128, 1], mm_dt, kind="ExternalInput")
    ones1d = nc.dram_tensor("ones1d", [1, 128], mm_dt, kind="ExternalInput")
    Y = nc.dram_tensor("Y", [ROWS, H], F32, kind="ExternalOutput")

    with tile.TileContext(nc) as tc:
        with tc.tile_pool(name="res", bufs=1) as res:
            qT_all = res.tile([128, KT, ROWS], mm_dt)
            attnT_all = res.tile([128, NH, ROWS], mm_dt)
            maskT_all = res.tile([128, JT, ROWS], mask_dt)
            bq_all = res.tile([128, KT, 1], F32)
            nc.sync.dma_start(bq_all, bq[:, :].rearrange("(t p) x -> p t x", p=128))
            bpB_all = res.tile([128, H], F32)
            nc.sync.dma_start(bpB_all, bpB[:, :])
            ones_sb = res.tile([128, 1], mm_dt)
            nc.sync.dma_start(ones_sb, onesd[:, :])
            ones1_sb = res.tile([1, 128], mm_dt)
            nc.sync.dma_start(ones1_sb, ones1d[:, :])

            wpp = tc.alloc_tile_pool(name="wpp", bufs=cfg["wpp"])
            kv = tc.alloc_tile_pool(name="kv", bufs=cfg["kv"])
            tp = tc.alloc_tile_pool(name="tp", bufs=cfg["tp"])
            pp = tc.alloc_tile_pool(name="pp", bufs=cfg["pp"])
            ps_s = tc.alloc_tile_pool(name="ps_s", bufs=cfg["scb"], space="PSUM")
            ps_z = tc.alloc_tile_pool(name="ps_z", bufs=cfg["zpb"], space="PSUM")
            ps_o = tc.alloc_tile_pool(name="ps_o", bufs=cfg["opb"], space="PSUM")

            with tc.tile_pool(name="p1", bufs=1) as p1, \
                 tc.tile_pool(name="p1w", bufs=cfg["p1w"]) as p1w, \
                 tc.tile_pool(name="ps_q", bufs=2, space="PSUM") as ps_q:
                xT_all = p1.tile([128, KT, ROWS], mm_dt)
                xT_ap = xT[:, :].rearrange("(t p) i -> p t i", p=128)
                for k in range(KT):
                    nc.sync.dma_start(xT_all[:, k, :], xT_ap[:, k, :])
                wqT_ap = wqT[:, :].rearrange("(a p) o -> p a o", p=128)
                for t in range(KT):
                    w_sb = p1w.tile([128, KT, 128], mm_dt, tag="wq")
                    nc.sync.dma_start(w_sb[:, :KT // 2, :],
                                      wqT_ap[:, :KT // 2, 128 * t:128 * (t + 1)])
                    nc.sync.dma_start(w_sb[:, KT // 2:, :],
                                      wqT_ap[:, KT // 2:, 128 * t:128 * (t + 1)])
                    psq = ps_q.tile([128, ROWS], F32, tag="psq")
                    for k in range(KT):
                        nc.tensor.matmul(psq, w_sb[:, k, :], xT_all[:, k, :],
                                         start=(k == 0), stop=(k == KT - 1))
                    nc.scalar.activation(qT_all[:, t, :], psq, IDENT,
                                         bias=bq_all[:, t, :])

            sm = tc.alloc_tile_pool(name="sm", bufs=2)
            maskT_ap = maskT[:, :].rearrange("(t p) i -> p t i", p=128)
            for j in range(JT):
                nc.sync.dma_start(maskT_all[:, j, :], maskT_ap[:, j, :])
            for h in range(NH):
                k_sbs, v_sbs = [], []
                for hf in range(2):
                    k_sb = kv.tile([128, JT // 2, 128], mm_dt, tag="k",
                                   name=f"k{h}_{hf}")
                    nc.sync.dma_start(
                        k_sb, key[h, :, 1024 * hf:1024 * (hf + 1)]
                        .rearrange("d (a j) -> d a j", j=128))
                    v_sb = kv.tile([128, JT // 2, 128], mm_dt, tag="v",
                                   name=f"v{h}_{hf}")
                    nc.sync.dma_start(
                        v_sb, value[h, 1024 * hf:1024 * (hf + 1), :]
                        .rearrange("(a p) d -> p a d", p=128))
                    k_sbs.append(k_sb)
                    v_sbs.append(v_sb)

                zp = ps_z.tile([1, ROWS], F32, tag="z")
                op = ps_o.tile([128, ROWS], F32, tag="o")
                pend = []

                def consume(gp, p_tile):
                    for uu in range(p_tile.shape[1]):
                        jtc = GS * gp + uu
                        nc.tensor.matmul(op, v_sbs[jtc // 8][:, jtc % 8, :],
                                         p_tile[:, uu, :],
                                         start=(jtc == 0), stop=(jtc == JT - 1))
                        nc.tensor.matmul(zp, ones_sb, p_tile[:, uu, :],
                                         start=(jtc == 0), stop=(jtc == JT - 1))

                for gidx in range(JT // GS):
                    W = ROWS
                    sc = ps_s.tile([128, GS * W], F32, tag="s", name=f"sc{h}_{gidx}")
                    t_sb = tp.tile([128, GS, W], F32, tag="t", name=f"t{h}_{gidx}")
                    for u in range(GS):
                        jt = GS * gidx + u
                        nc.tensor.matmul(sc[:, W * u:W * (u + 1)],
                                         k_sbs[jt // 8][:, jt % 8, :],
                                         qT_all[:, h, :], start=True, stop=True)
                        nc.vector.scalar_tensor_tensor(
                            t_sb[:, u, :], sc[:, W * u:W * (u + 1)],
                            1.0, maskT_all[:, jt, :], MULT, ADD)
                    p_sb = pp.tile([128, GS, W], mm_dt, tag="p", name=f"p{h}_{gidx}")
                    nc.scalar.activation(p_sb, t_sb, EXP, scale=SCALE)
                    pend.append((gidx, p_sb))
                    if len(pend) > 1:
                        consume(*pend.pop(0))
                while pend:
                    consume(*pend.pop(0))

                rc = sm.tile([1, ROWS], mm_dt, tag="rc")
                with nc.allow_low_precision(reason="f32r reciprocal storage"):
                    nc.vector.reciprocal(rc, zp)
                bc = ps_s.tile([128, ROWS], F32, tag="s")
                nc.tensor.matmul(bc, ones1_sb, rc, start=True, stop=True)
                rb = sm.tile([128, ROWS], F32, tag="rb")
                nc.scalar.copy(rb, bc)
                nc.vector.tensor_tensor(attnT_all[:, h, :], op, rb, op=MULT)

            sm.release()
            ps_o.release()
            ps_z.release()
            ps_s.release()
            pp.release()
            tp.release()
            kv.release()

            with tc.tile_pool(name="ypo", bufs=2) as ypo, \
                 tc.tile_pool(name="ps_y", bufs=4, space="PSUM") as ps_y:
                wpT_ap = wpT[:, :].rearrange("(a p) o -> p a o", p=128)
                for half in range(2):
                    o0 = 1024 * half
                    psys = []
                    for it in range(IT):
                        psy = ps_y.tile([128, 1024], F32, tag="y",
                                        name=f"psy{half}_{it}")
                        psys.append(psy)
                    for k in range(KT):
                        wp_sb = wpp.tile([128, 1024], mm_dt, tag="wp")
                        nc.sync.dma_start(wp_sb, wpT_ap[:, k, o0:o0 + 1024])
                        for it in range(IT):
                            att = attnT_all[:, k, 128 * it:128 * (it + 1)]
                            for nb in range(2):
                                nc.tensor.matmul(
                                    psys[it][:, 512 * nb:512 * (nb + 1)],
                                    att, wp_sb[:, 512 * nb:512 * (nb + 1)],
                                    start=(k == 0), stop=(k == KT - 1))
                    for it in range(IT):
                        y_sb = ypo.tile([128, 1024], F32, tag="ysb")
                        nc.vector.tensor_tensor(y_sb, psys[it],
                                                bpB_all[:, o0:o0 + 1024], op=ADD)
                        nc.sync.dma_start(
                            Y[128 * it:128 * (it + 1), o0:o0 + 1024], y_sb)
            wpp.release()

    nc.compile()
    return nc


_CACHE = {}


def _get_nc(mm_dt=None, mask_dt=None, causal=True, cfg=None):
    """test.py compatibility: causal=True returns the bf16 causal kernel
    (mm_dt/mask_dt ignored); causal=False returns the general kernel."""
    if causal:
        ck = ("causal", tuple(sorted((cfg or {}).items())))
        if ck not in _CACHE:
            _CACHE[ck] = build_causal(cfg)
        return _CACHE[ck]
    ck = ("gen", str(mm_dt), str(mask_dt))
    if ck not in _CACHE:
        _CACHE[ck] = build_general(mm_dt if mm_dt is not None else F32R,
                                   mask_dt if mask_dt is not None else F32)
    return _CACHE[ck]


def _is_causal(attention_mask):
    """True if the mask is exactly the standard causal additive mask."""
    m = attention_mask
    if m.shape != (B, 1, SQ, SK):
        return False
    m0 = np.asarray(m[0, 0])
    tri = np.tril(np.ones((SQ, SK), dtype=bool))
    ref = np.where(tri, np.float32(0.0), np.float32(NEG))
    if not np.array_equal(m0, ref):
        return False
    for b in range(1, B):
        if not np.array_equal(np.asarray(m[b, 0]), m0):
            return False
    return True


def _kernel_causal(hidden_states, key, value, w_q, b_q, w_proj, b_proj,
                   _cfg=None, _trace=False):
    import ml_dtypes
    bf16 = ml_dtypes.bfloat16
    nc = _get_nc(causal=True, cfg=_cfg)

    wqT = np.ascontiguousarray(w_q.T).astype(bf16)
    wpT = np.ascontiguousarray(w_proj.T).astype(bf16)
    bq2 = np.ascontiguousarray(b_q[:, None]).astype(np.float32)
    bp1 = np.ascontiguousarray(b_proj[None, :]).astype(bf16)
    key_b = [np.ascontiguousarray(key[b * NH:(b + 1) * NH]).astype(bf16)
             for b in range(B)]
    # vR[h, p, a*HD + d] = v[h, a*128 + p, d]  (4KB contiguous runs per line)
    vR_b = [np.ascontiguousarray(
        value[b].reshape(NH, SK // 128, 128, HD).transpose(0, 2, 1, 3)
        .reshape(NH, 128, SK)).astype(bf16) for b in range(B)]
    inv_scale = np.float32(1.0 / SCALE)

    # band mask: for key-tile jt, col u of the 32-col band (query row
    # 128*jt + 4*u + s, key 128*jt + p): unmasked iff p <= 4*u + s
    p_idx = np.arange(128)[:, None]
    u_idx = np.arange(32)[None, :]

    in_maps = []
    for c in range(NCORES):
        b, s = c // 4, c % 4
        rows = s + 4 * np.arange(ROWS)
        xT_c = np.ascontiguousarray(hidden_states[b, rows, :].T).astype(bf16)
        # multiplicative post-exp masks: 1 keep, 0 drop
        band_c = (p_idx <= 4 * u_idx + s).astype(np.float32)
        band2 = np.ascontiguousarray(
            np.concatenate([band_c, band_c], axis=1)).astype(bf16)
        # bandN [128, (u, jj, 64)]: jj=0 -> [band | 1]; jj=1 -> [0 | band]
        one = np.ones((128, 32), np.float32)
        zer = np.zeros((128, 32), np.float32)
        row = np.concatenate([band_c, one, zer, band_c], axis=1)  # [128, 128]
        bandN_c = np.ascontiguousarray(
            np.concatenate([row, row], axis=1)).astype(bf16)  # [128, 256]
        in_maps.append(dict(
            xT=xT_c, wqT=wqT, bq=bq2, key=key_b[b], vR=vR_b[b],
            band=band2, bandN=bandN_c, wpT=wpT, bp1=bp1,
        ))

    kw = {}
    if _trace:
        kw = dict(trace=True, trace_cores=list(range(NCORES)), stitch_traces=False)
    res = run_bass_kernel_spmd(nc, in_maps, core_ids=list(range(NCORES)), **kw)
    if _trace:
        kernel._last_result = res

    out = np.empty((B, SQ, H), dtype=np.float32)
    for c in range(NCORES):
        b, s = c // 4, c % 4
        rows = s + 4 * np.arange(ROWS)
        out[b, rows, :] = res.results[c]["Y"]
    return out


def _kernel_general(hidden_states, key, value, attention_mask,
                    w_q, b_q, w_proj, b_proj, _mm_dt=F32R):
    nc = _get_nc(_mm_dt, F32, causal=False)
    wqT = np.ascontiguousarray(w_q.T)
    wpT = np.ascontiguousarray(w_proj.T)
    bq2 = np.ascontiguousarray(b_q[:, None]).astype(np.float32)
    bpB = np.ascontiguousarray(
        np.broadcast_to(b_proj[None, :], (128, H))).astype(np.float32)
    key_b = [np.ascontiguousarray(key[b * NH:(b + 1) * NH]) for b in range(B)]
    val_b = [np.ascontiguousarray(value[b]) for b in range(B)]
    inv_scale = np.float32(1.0 / SCALE)

    in_maps = []
    for c in range(NCORES):
        b, sидx = c // 4, c % 4
        rows = np.arange(ROWS * sидx, ROWS * sидx + ROWS)
        xT_c = np.ascontiguousarray(hidden_states[b, rows, :].T)
        maskT_c = np.ascontiguousarray(
            (attention_mask[b, 0, rows, :].T * inv_scale).astype(np.float32))
        in_maps.append(dict(
            xT=xT_c, wqT=wqT, bq=bq2, key=key_b[b], value=val_b[b],
            maskT=maskT_c, wpT=wpT, bpB=bpB,
            onesd=np.ones((128, 1), dtype=np.float32),
            ones1d=np.ones((1, 128), dtype=np.float32),
        ))
    res = run_bass_kernel_spmd(nc, in_maps, core_ids=list(range(NCORES)))
    out = np.empty((B, SQ, H), dtype=np.float32)
    for c in range(NCORES):
        b, sидx = c // 4, c % 4
        rows = np.arange(ROWS * sидx, ROWS * sидx + ROWS)
        out[b, rows, :] = res.results[c]["Y"]
    return out


def kernel(hidden_states, key, value, attention_mask, w_q, b_q, w_proj, b_proj,
           _mm_dt=F32R, _trace=False, _cfg=None):
    hidden_states = np.asarray(hidden_states)
    key = np.asarray(key)
    value = np.asarray(value)
    attention_mask = np.asarray(attention_mask)
    w_q = np.asarray(w_q)
    b_q = np.asarray(b_q)
    w_proj = np.asarray(w_proj)
    b_proj = np.asarray(b_proj)

    if _is_causal(attention_mask):
        return _kernel_causal(hidden_states, key, value, w_q, b_q,
                              w_proj, b_proj, _cfg=_cfg, _trace=_trace)
    return _kernel_general(hidden_states, key, value, attention_mask,
                           w_q, b_q, w_proj, b_proj, _mm_dt=_mm_dt)


BF16_ = BF16  # back-compat alias


if __name__ == "__main__":
    pass
